# revision 4
# baseline (speedup 1.0000x reference)
"""Trainium2 Bass kernel for the Binary-MLP (nn_Binary0) problem.

Strategy (8-way batch-parallel, 1024 rows/core):
  fc1: h1 = x @ sign(w1).T        -- fp16x2 split of x (exact to ~2^-22):
       pass1 rhs = fp16(x), lhsT = +-1; pass2 rhs = fp16((x-x1)*2^11),
       lhsT = +-2^-11 (both weight scales exact in fp8e5m2). 13 k-tiles
       vs 19 for the old bf16x3 split. k-outer loop over j-groups of 3
       so the PE saturates as soon as the first x k-tiles land.
       a1 = sign(h1 - t1)          -- thresholds fold bias+BN (host fp64)
  fc2: h2 = a1 @ sign(w2).T        -- fp8 DoubleRow (exact: +-1 products)
       a2 = sign(h2 - t2)
  fc3: h3 = a2 @ sign(w3).T        -- fp8 DoubleRow
       h3c = clip(h3*s3 + c3, -1, 1) -> fp16
  fc4: logits.T = w4 @ h3c         -- fp16 (216ns/MM vs 380 for f32r),
                                      fused into fc3 loop, [cls, batch]
  out = log_softmax(logits)        -- PE-transpose, free-dim reduce,
                                      single batched output DMA

DMA: triggers cost ~650ns serially per queue (~200GB/s per queue for
128KB transfers) -> few big chunked DMAs, split across the sync and
scalar (Activation) hardware queues at startup.
"""
import sys

for _p in ("/opt/trn_rl_repo",):
    if _p not in sys.path:
        sys.path.insert(0, _p)

import numpy as np
import ml_dtypes

import concourse.bass as bass
import concourse.tile as tile
import concourse.mybir as mybir
from concourse.bass_utils import run_bass_kernel_spmd
from concourse.masks import make_identity

F32 = mybir.dt.float32
F16 = mybir.dt.float16
BF16 = mybir.dt.bfloat16
FP8 = mybir.dt.float8e4
FP8E5 = mybir.dt.float8e5
NP_FP8 = mybir.dt.np(FP8)
NP_FP8E5 = mybir.dt.np(FP8E5)

EPS = 1e-5
NCORES = 8
B = 8192
BC = B // NCORES            # 1024 batch rows per core
D0, D1, D2 = 784, 3072, 6144
K1 = 13                     # fc1 k-tiles: 6 pass1 + 6 pass2 + 1 packed
K1F = 6                     # full 128-row k-tiles per pass (768 rows)
NJ1 = D1 // 128             # 24 fc1 output feature tiles
G1 = 3                      # fc1 j-tiles per psum group
NG1 = NJ1 // G1             # 8 groups
NT2 = D1 // 256             # 12 fc2 DoubleRow contraction tiles
NJ2 = D2 // 128             # 48
NT3 = D2 // 256             # 24 fc3 DoubleRow contraction tiles
NJ3 = D2 // 128             # 48
JB = 4                      # j-tiles per streamed weight slab
NB = 2                      # 512-wide batch halves of BC
NBCH = BC // 128            # 8 batch chunks
NCLS = 16                   # padded class dim (10 real)
S2L = 2.0 ** 11             # pass2 rhs scale
S2W = 2.0 ** -11            # pass2 weight scale

TRACE = False               # test.py sets True for profiling
TRACE_DIR = None
LAST_EXEC_NS = None

DR = mybir.MatmulPerfMode.DoubleRow
ACTF = mybir.ActivationFunctionType
ALU = mybir.AluOpType


def _legalize_multiwait(nc):
    """This container's walrus build rejects >1 sync-wait on one instruction
    (codegen 'Too many sync wait commands'); split extra waits into NoOps."""
    n = 0
    for f in nc.m.functions:
        for blk in f.blocks:
            insts = list(blk.instructions)
            new = []
            changed = False
            for ins in insts:
                si = ins.sync_info
                waits = list(si.on_wait) if (si is not None and si.on_wait) else []
                if len(waits) > 1:
                    for k, w in enumerate(waits[:-1]):
                        nop = mybir.InstNoOp(name=f"{ins.name}-sw{k}", ins=[], outs=[])
                        nop.engine = ins.engine
                        nop.sync_info = mybir.SyncInfo(on_wait=[w], on_update=[])
                        new.append(nop)
                        n += 1
                    ins.sync_info = mybir.SyncInfo(
                        on_wait=[waits[-1]], on_update=list(si.on_update or [])
                    )
                    changed = True
                new.append(ins)
            if changed:
                blk.instructions = new
    return n


def _build_nc():
    nc = bass.Bass("TRN2")

    xht = nc.dram_tensor("xht", [128, K1 * BC], F16, kind="ExternalInput")
    w1t = nc.dram_tensor("w1t", [128, NG1 * K1 * G1 * 128], FP8E5,
                         kind="ExternalInput")
    w2p = nc.dram_tensor("w2p", [NJ2 // JB, 128, NT2 * 2 * JB * 128], FP8,
                         kind="ExternalInput")
    w3p = nc.dram_tensor("w3p", [NJ3 // JB, 128, NT3 * 2 * JB * 128], FP8,
                         kind="ExternalInput")
    w4t = nc.dram_tensor("w4t", [128, NJ3 * NCLS], F16, kind="ExternalInput")
    # cvec columns: [0:24]=-t1, [24:72]=-t2, [72:120]=s3, [120:168]=c3
    cvec = nc.dram_tensor("cvec", [128, NJ1 + 3 * NJ3], F32, kind="ExternalInput")
    b4c = nc.dram_tensor("b4c", [NCLS, 1], F32, kind="ExternalInput")
    out = nc.dram_tensor("out", [BC, 10], F32, kind="ExternalOutput")

    xr = xht.rearrange("p (k c) -> p k c", c=BC)
    wr = w1t.rearrange("p (g k c) -> p g k c", k=K1, c=G1 * 128)

    with tile.TileContext(nc) as tc:
        with (
            tc.tile_pool(name="consts", bufs=1) as consts,
            tc.tile_pool(name="a1p", bufs=1) as a1p,
            tc.tile_pool(name="a2p", bufs=1) as a2p,
            tc.tile_pool(name="psum", bufs=5, space="PSUM") as psum,
            tc.tile_pool(name="psum_lg", bufs=2, space="PSUM") as psum_lg,
            tc.tile_pool(name="psum_tp", bufs=1, space="PSUM") as psum_tp,
        ):
            a1 = a1p.tile([128, NT2, 2, BC], FP8)
            a2 = a2p.tile([128, NT3, 2, BC], FP8)

            # fc4 logits accumulators [cls, batch-half], pre-zeroed, start=False
            lg = [psum_lg.tile([NCLS, 512], F32, tag="lg", name=f"lg{i}")
                  for i in range(NB)]

            # ---- fc1: fp16x2 exact split + sign threshold ----
            with tc.tile_pool(name="fc1res", bufs=1) as fc1res:
                xh = fc1res.tile([128, K1, BC], F16)
                w1s = fc1res.tile([128, NG1, K1, G1 * 128], FP8E5)

                # startup-critical DMAs, split across the sync + scalar
                # hardware queues; x n=0 chunks pace the first j-group.
                nc.sync.dma_start(out=xh[:, 0:3, 0:512], in_=xr[:, 0:3, 0:512])
                nc.sync.dma_start(out=xh[:, 3:6, 0:512], in_=xr[:, 3:6, 0:512])
                nc.sync.dma_start(out=xh[:, 6:9, 0:512], in_=xr[:, 6:9, 0:512])
                nc.sync.dma_start(out=xh[:, 9:K1, 0:512], in_=xr[:, 9:K1, 0:512])
                for g in range(NG1):
                    nc.scalar.dma_start(out=w1s[:, g], in_=wr[:, g])
                cv = consts.tile([128, NJ1 + 3 * NJ3], F32)
                nc.sync.dma_start(out=cv, in_=cvec[:, :])
                nt1 = cv[:, 0:NJ1]
                nt2 = cv[:, NJ1:NJ1 + NJ3]
                s3s = cv[:, NJ1 + NJ3:NJ1 + 2 * NJ3]
                c3s = cv[:, NJ1 + 2 * NJ3:NJ1 + 3 * NJ3]
                w4s = consts.tile([128, NJ3, NCLS], F16)
                nc.sync.dma_start(
                    out=w4s, in_=w4t.rearrange("p (j c) -> p j c", c=NCLS))
                b4s = consts.tile([NCLS, 1], F32)
                nc.sync.dma_start(out=b4s, in_=b4c[:, :])
                # x n=1 half: needed only after phase 0 (~65us in)
                nc.sync.dma_start(out=xh[:, 0:7, 512:1024],
                                  in_=xr[:, 0:7, 512:1024])
                nc.sync.dma_start(out=xh[:, 7:K1, 512:1024],
                                  in_=xr[:, 7:K1, 512:1024])
                for n in range(NB):
                    nc.vector.memset(lg[n], 0.0)

                for n in range(NB):
                    for g in range(NG1):
                        pss = [psum.tile([128, 512], F32, tag="ps",
                                         name=f"f1_{n}_{g}_{i}")
                               for i in range(G1)]
                        for k in range(K1):
                            for j3 in range(G1):
                                nc.tensor.matmul(
                                    pss[j3],
                                    lhsT=w1s[:, g, k, j3 * 128:(j3 + 1) * 128],
                                    rhs=xh[:, k, n * 512:(n + 1) * 512],
                                    start=(k == 0),
                                    stop=(k == K1 - 1),
                                )
                        for j3 in range(G1):
                            j = g * G1 + j3
                            nc.scalar.activation(
                                out=a1[:, j // 2, j % 2, n * 512:(n + 1) * 512],
                                in_=pss[j3],
                                func=ACTF.Sign,
                                bias=nt1[:, j:j + 1],
                                scale=1.0,
                            )

            # ---- fc2: fp8 DoubleRow + sign threshold ----
            with tc.tile_pool(name="w2s", bufs=2) as w2s:
                for jb in range(NJ2 // JB):
                    wt = w2s.tile([128, NT2, 2, JB * 128], FP8, tag="w2t")
                    nc.sync.dma_start(
                        out=wt,
                        in_=w2p[jb].rearrange("p (t i c) -> p t i c",
                                              i=2, c=JB * 128),
                    )
                    for j in range(JB):
                        jj = jb * JB + j
                        for n in range(NB):
                            ps = psum.tile([128, 512], F32, tag="ps")
                            for t in range(NT2):
                                nc.tensor.matmul(
                                    ps,
                                    lhsT=wt[:, t, :, j * 128:(j + 1) * 128],
                                    rhs=a1[:, t, :, n * 512:(n + 1) * 512],
                                    start=(t == 0),
                                    stop=(t == NT2 - 1),
                                    perf_mode=DR,
                                )
                            nc.scalar.activation(
                                out=a2[:, jj // 2, jj % 2, n * 512:(n + 1) * 512],
                                in_=ps,
                                func=ACTF.Sign,
                                bias=nt2[:, jj:jj + 1],
                                scale=1.0,
                            )

            # ---- fc3 (fp8 DoubleRow) + bn3/hardtanh + fused fc4 (fp16) ----
            with (
                tc.tile_pool(name="w3s", bufs=2) as w3s,
                tc.tile_pool(name="h3p", bufs=3) as h3p,
            ):
                for jb in range(NJ3 // JB):
                    wt = w3s.tile([128, NT3, 2, JB * 128], FP8, tag="w3t")
                    nc.sync.dma_start(
                        out=wt,
                        in_=w3p[jb].rearrange("p (t i c) -> p t i c",
                                              i=2, c=JB * 128),
                    )
                    for j in range(JB):
                        jj = jb * JB + j
                        h3 = h3p.tile([128, BC], F16, tag="h3")
                        for n in range(NB):
                            ps = psum.tile([128, 512], F32, tag="ps")
                            for t in range(NT3):
                                nc.tensor.matmul(
                                    ps,
                                    lhsT=wt[:, t, :, j * 128:(j + 1) * 128],
                                    rhs=a2[:, t, :, n * 512:(n + 1) * 512],
                                    start=(t == 0),
                                    stop=(t == NT3 - 1),
                                    perf_mode=DR,
                                )
                            tmp = h3p.tile([128, 512], F32, tag="bn3tmp")
                            nc.scalar.activation(
                                out=tmp,
                                in_=ps,
                                func=ACTF.Identity,
                                bias=c3s[:, jj:jj + 1],
                                scale=s3s[:, jj:jj + 1],
                            )
                            nc.vector.tensor_scalar(
                                out=h3[:, n * 512:(n + 1) * 512],
                                in0=tmp,
                                scalar1=-1.0,
                                scalar2=1.0,
                                op0=ALU.max,
                                op1=ALU.min,
                            )
                            # fused fc4 (fp16): lg[n][c, b] += w4[c,:] @ h3c[:, b]
                            nc.tensor.matmul(
                                lg[n],
                                lhsT=w4s[:, jj, :],
                                rhs=h3[:, n * 512:(n + 1) * 512],
                                start=False,
                                stop=(jj == NJ3 - 1),
                                skip_group_check=True,
                            )

            # ---- epilogue: +b4, transpose [cls,b]->[b,cls], log_softmax ----
            with tc.tile_pool(name="epi", bufs=1) as epi:
                ident = consts.tile([NCLS, NCLS], F32)
                make_identity(nc, ident)
                lsb = epi.tile([NCLS, BC], F32, tag="lsb")
                tp = psum_tp.tile([128, NBCH, NCLS], F32, tag="tp")
                for n in range(NB):
                    nc.scalar.activation(
                        out=lsb[:, n * 512:(n + 1) * 512],
                        in_=lg[n],
                        func=ACTF.Identity,
                        bias=b4s[:, 0:1],
                        scale=1.0,
                    )
                    for b in range(4):
                        c = n * 4 + b
                        nc.tensor.transpose(
                            tp[:, c, :], lsb[:, c * 128:(c + 1) * 128], ident)
                # log_softmax without max-shift: logits are O(5), exp safe
                ex = epi.tile([128, NBCH, 10], F32, tag="ex")
                nc.scalar.activation(out=ex, in_=tp[:, :, 0:10], func=ACTF.Exp)
                sm = epi.tile([128, NBCH], F32, tag="sm")
                nc.vector.tensor_reduce(
                    out=sm, in_=ex, axis=mybir.AxisListType.X, op=ALU.add)
                lnt = epi.tile([128, NBCH], F32, tag="lnt")
                nc.scalar.activation(out=lnt, in_=sm, func=ACTF.Ln)
                res = epi.tile([128, NBCH, 10], F32, tag="res")
                for b in range(NBCH):
                    nc.vector.tensor_scalar(
                        out=res[:, b, :], in0=tp[:, b, 0:10],
                        scalar1=lnt[:, b:b + 1],
                        scalar2=None, op0=ALU.subtract,
                    )
                nc.sync.dma_start(
                    out=out.rearrange("(c p) f -> p c f", p=128), in_=res)

    _legalize_multiwait(nc)
    return nc


def _prep_inputs(inputs):
    f64 = {k: np.asarray(v, np.float64) for k, v in inputs.items()
           if k != "x"}
    x = np.asarray(inputs["x"], np.float32)

    s1 = f64["g1"] / np.sqrt(f64["v1"] + EPS)
    t1 = f64["m1"] - f64["b1"] - f64["be1"] / s1
    s2 = f64["g2"] / np.sqrt(f64["v2"] + EPS)
    t2 = f64["m2"] - f64["b2"] - f64["be2"] / s2
    s3 = f64["g3"] / np.sqrt(f64["v3"] + EPS)
    c3 = (f64["b3"] - f64["m3"]) * s3 + f64["be3"]

    shared = {}
    # cvec [128, 24+48*3]: per-feature consts arranged [partition, tile]
    cvec = np.zeros((128, NJ1 + 3 * NJ3), np.float32)
    cvec[:, 0:NJ1] = (-t1).astype(np.float32).reshape(NJ1, 128).T
    cvec[:, NJ1:NJ1 + NJ3] = (-t2).astype(np.float32).reshape(NJ3, 128).T
    cvec[:, NJ1 + NJ3:NJ1 + 2 * NJ3] = s3.astype(np.float32).reshape(NJ3, 128).T
    cvec[:, NJ1 + 2 * NJ3:] = c3.astype(np.float32).reshape(NJ3, 128).T
    shared["cvec"] = np.ascontiguousarray(cvec)

    b4p = np.zeros((NCLS, 1), np.float32)
    b4p[:10, 0] = np.asarray(inputs["b4"], np.float32)
    shared["b4c"] = b4p

    # w1: sign, transposed to [in, out]; k-tiles 0-5 = pass1 rows (+-1),
    # 6-11 = pass2 rows (+-2^-11), 12 = packed remainders of both passes.
    # Then permuted to j-group-major so each group is one contiguous DMA.
    w1b = np.sign(np.asarray(inputs["w1"], np.float32)).astype(np.float32)
    w1T = w1b.T  # [784, D1]
    w1f = np.zeros((128, K1, D1), np.float32)
    for k in range(K1F):
        w1f[:, k, :] = w1T[k * 128:(k + 1) * 128]
        w1f[:, k + K1F, :] = w1T[k * 128:(k + 1) * 128] * S2W
    w1f[0:16, 12, :] = w1T[768:784]
    w1f[16:32, 12, :] = w1T[768:784] * S2W
    w1e5 = w1f.astype(NP_FP8E5)
    shared["w1t"] = np.ascontiguousarray(
        w1e5.reshape(128, K1, NG1, G1 * 128).transpose(0, 2, 1, 3)
        .reshape(128, NG1 * K1 * G1 * 128))

    # w2/w3: sign -> DoubleRow pair layout, slab-contiguous per partition:
    # [njb, 128, nt*2*(JB*128)] fp8
    def pack_dr(w, njb_out):
        wT = np.sign(np.asarray(w, np.float32)).T  # [in, out]
        nin, nout = wT.shape
        nt = nin // 256
        a = wT.reshape(nt, 2, 128, nout).transpose(0, 2, 1, 3)  # [nt,128,2,out]
        a = a.reshape(nt, 128, 2, njb_out, JB * 128).transpose(3, 1, 0, 2, 4)
        # a: [njb, 128, nt, 2, JB*128]
        return np.ascontiguousarray(
            a.reshape(njb_out, 128, nt * 2 * JB * 128).astype(NP_FP8))

    shared["w2p"] = pack_dr(inputs["w2"], NJ2 // JB)
    shared["w3p"] = pack_dr(inputs["w3"], NJ3 // JB)

    # w4: [10, D2] -> fp16 [128, NJ3*NCLS]: elem [k, j*16+c] = w4[c, j*128+k]
    w4 = np.asarray(inputs["w4"], np.float32)
    w4tp = np.zeros((D2, NCLS), np.float32)
    w4tp[:, :10] = w4.T
    shared["w4t"] = np.ascontiguousarray(
        w4tp.reshape(NJ3, 128, NCLS).transpose(1, 0, 2)
        .reshape(128, NJ3 * NCLS).astype(np.float16))

    # x: transpose, fp16x2 split (pass2 scaled by 2^11); per-core layout
    # [128, K1*BC] with k-tile-major columns.
    xT = np.ascontiguousarray(x.T)  # [784, B]
    x1 = xT.astype(np.float16)
    x2s = ((xT - x1.astype(np.float32)) * S2L).astype(np.float16)
    per_core = []
    for cix in range(NCORES):
        sl = slice(cix * BC, (cix + 1) * BC)
        xa = np.zeros((K1, 128, BC), np.float16)
        xa[0:K1F] = x1[0:768, sl].reshape(K1F, 128, BC)
        xa[K1F:2 * K1F] = x2s[0:768, sl].reshape(K1F, 128, BC)
        xa[12, 0:16] = x1[768:784, sl]
        xa[12, 16:32] = x2s[768:784, sl]
        m = dict(shared)
        m["xht"] = np.ascontiguousarray(
            xa.transpose(1, 0, 2).reshape(128, K1 * BC))
        per_core.append(m)
    return per_core


_NC_CACHE = None


def kernel(**inputs):
    global _NC_CACHE, LAST_EXEC_NS
    if _NC_CACHE is None:
        _NC_CACHE = _build_nc()
    nc = _NC_CACHE
    in_maps = _prep_inputs(inputs)
    kwargs = {}
    if TRACE:
        _install_ntff_shim()
        kwargs = dict(trace=True, tmpdir=TRACE_DIR)
    res = None
    outs = None
    for attempt in range(3):
        try:
            res = run_bass_kernel_spmd(nc, in_maps, core_ids=list(range(NCORES)),
                                       **kwargs)
            outs = [np.asarray(res.results[c]["out"]) for c in range(NCORES)]
            break
        except Exception:
            if attempt == 2:
                raise
    LAST_EXEC_NS = res.exec_time_ns
    return np.concatenate(outs, axis=0)


def _install_ntff_shim():
    """antenv.axon_hooks shim so trace=True works under axon (profiling only)."""
    import contextlib
    import ctypes
    import types

    if "antenv.axon_hooks" in sys.modules:
        return
    try:
        lib = ctypes.CDLL("/opt/axon/libaxon_pjrt.so")
        lib.axon_start_nrt_profile.argtypes = [
            ctypes.POINTER(ctypes.c_int64), ctypes.c_size_t]
        lib.axon_start_nrt_profile.restype = ctypes.c_int64
        lib.axon_stop_nrt_profile.argtypes = [ctypes.c_char_p]
        lib.axon_stop_nrt_profile.restype = ctypes.c_int64
    except (OSError, AttributeError):
        return

    @contextlib.contextmanager
    def _hook(output_dir, device_ids):
        import jax
        jax.devices()
        if device_ids:
            ids = (ctypes.c_int64 * len(device_ids))(*device_ids)
            rc = lib.axon_start_nrt_profile(ids, len(device_ids))
        else:
            rc = lib.axon_start_nrt_profile(None, 0)
        if rc != 0:
            raise RuntimeError(f"axon_start_nrt_profile rc={rc}")
        try:
            yield
        finally:
            n = lib.axon_stop_nrt_profile(str(output_dir).encode())
            print(f"ntff: {n} profile file(s) -> {output_dir}", file=sys.stderr)

    mod = types.ModuleType("antenv.axon_hooks")
    mod.get_axon_ntff_profile_hook = lambda: _hook
    mod.set_axon_ntff_profile_hook = lambda h: None
    sys.modules["antenv.axon_hooks"] = mod


# revision 5
# speedup vs baseline: 1.1670x; 1.1670x over previous
"""Trainium2 Bass kernel for the Binary-MLP (nn_Binary0) problem.

Strategy (8-way batch-parallel, 1024 rows/core):
  fc1: h1 = x @ sign(w1).T        -- bf16x3 split of x (fp32-exact), bf16 +-1 weights
       a1 = sign(h1 - t1)          -- thresholds fold bias+BN (host fp64), fp8 out
  fc2: h2 = a1 @ sign(w2).T        -- fp8 DoubleRow (exact: +-1 products, fp32 psum)
       a2 = sign(h2 - t2)
  fc3: h3 = a2 @ sign(w3).T        -- fp8 DoubleRow
       h3c = clip(h3*s3 + c3, -1, 1)
  fc4: logits.T = w4 @ h3c         -- fp32r (tf32-class, ample for fc4), fused into
                                      fc3 loop, [cls, batch] psum accumulation
  out = log_softmax(logits)        -- PE-transpose to [batch, cls], free-dim reduce

All activations live feature-major [feature, batch] so per-feature thresholds
are per-partition ACT bias vectors, and each layer's sign-output writes land
directly in the DoubleRow-paired [k, 2, batch] slab layout the next layer needs.
"""
import sys

for _p in ("/opt/trn_rl_repo",):
    if _p not in sys.path:
        sys.path.insert(0, _p)

import numpy as np
import ml_dtypes

import concourse.bass as bass
import concourse.tile as tile
import concourse.mybir as mybir
from concourse.bass_utils import run_bass_kernel_spmd
from concourse.masks import make_identity

F32 = mybir.dt.float32
F32R = mybir.dt.float32r
BF16 = mybir.dt.bfloat16
FP8 = mybir.dt.float8e4
NP_FP8 = mybir.dt.np(FP8)
NP_BF16 = ml_dtypes.bfloat16

EPS = 1e-5
NCORES = 8
B = 8192
BC = B // NCORES            # 1024 batch rows per core
D0, D1, D2 = 784, 3072, 6144
K1 = 7                      # fc1 k-tiles per group: 6 full + 1 packed remainder
K1F = 6                     # full 128-row k-tiles (768 rows)
NXT = 19                    # x3 sbuf tiles: 3 passes x 6 full + 1 packed
NPASS = 3                   # bf16 splits of x
NJ1 = D1 // 128             # 24 fc1 output feature tiles
NT2 = D1 // 256             # 12 fc2 DoubleRow contraction tiles
NJ2 = D2 // 128             # 48
NT3 = D2 // 256             # 24 fc3 DoubleRow contraction tiles
NJ3 = D2 // 128             # 48
JB = 4                      # j-tiles per streamed weight slab
NB = 2                      # 512-wide batch halves of BC
NBCH = BC // 128            # 8 batch chunks
NCLS = 16                   # padded class dim (10 real)

TRACE = False               # test.py sets True for profiling
TRACE_DIR = None
LAST_EXEC_NS = None

DR = mybir.MatmulPerfMode.DoubleRow
ACTF = mybir.ActivationFunctionType
ALU = mybir.AluOpType


def _legalize_multiwait(nc):
    """This container's walrus build rejects >1 sync-wait on one instruction
    (codegen 'Too many sync wait commands'); split extra waits into NoOps."""
    n = 0
    for f in nc.m.functions:
        for blk in f.blocks:
            insts = list(blk.instructions)
            new = []
            changed = False
            for ins in insts:
                si = ins.sync_info
                waits = list(si.on_wait) if (si is not None and si.on_wait) else []
                if len(waits) > 1:
                    for k, w in enumerate(waits[:-1]):
                        nop = mybir.InstNoOp(name=f"{ins.name}-sw{k}", ins=[], outs=[])
                        nop.engine = ins.engine
                        nop.sync_info = mybir.SyncInfo(on_wait=[w], on_update=[])
                        new.append(nop)
                        n += 1
                    ins.sync_info = mybir.SyncInfo(
                        on_wait=[waits[-1]], on_update=list(si.on_update or [])
                    )
                    changed = True
                new.append(ins)
            if changed:
                blk.instructions = new
    return n


def _build_nc():
    nc = bass.Bass("TRN2")

    x3t = nc.dram_tensor("x3t", [NXT, 128, BC], BF16, kind="ExternalInput")
    w1t = nc.dram_tensor("w1t", [K1, 128, D1], FP8, kind="ExternalInput")
    w2p = nc.dram_tensor("w2p", [NJ2 // JB, NT2, 128, 2, JB * 128], FP8,
                         kind="ExternalInput")
    w3p = nc.dram_tensor("w3p", [NJ3 // JB, NT3, 128, 2, JB * 128], FP8,
                         kind="ExternalInput")
    w4t = nc.dram_tensor("w4t", [128, NJ3 * NCLS], F32R, kind="ExternalInput")
    # cvec columns: [0:24]=-t1, [24:72]=-t2, [72:120]=s3, [120:168]=c3
    cvec = nc.dram_tensor("cvec", [128, NJ1 + 3 * NJ3], F32, kind="ExternalInput")
    b4c = nc.dram_tensor("b4c", [NCLS, 1], F32, kind="ExternalInput")
    out = nc.dram_tensor("out", [BC, 10], F32, kind="ExternalOutput")

    with tile.TileContext(nc) as tc:
        with (
            tc.tile_pool(name="consts", bufs=1) as consts,
            tc.tile_pool(name="a1p", bufs=1) as a1p,
            tc.tile_pool(name="a2p", bufs=1) as a2p,
            tc.tile_pool(name="psum", bufs=5, space="PSUM") as psum,
            tc.tile_pool(name="psum_lg", bufs=2, space="PSUM") as psum_lg,
            tc.tile_pool(name="psum_tp", bufs=1, space="PSUM") as psum_tp,
            tc.tile_pool(name="w2s", bufs=3) as w2s,
        ):
            a1 = a1p.tile([128, NT2, 2, BC], FP8)
            a2 = a2p.tile([128, NT3, 2, BC], FP8)

            # fc4 logits accumulators [cls, batch-half], pre-zeroed, start=False
            lg = [psum_lg.tile([NCLS, 512], F32, tag="lg", name=f"lg{i}")
                  for i in range(NB)]

            # ---- fc1: bf16x3 exact fp32 matmul + sign threshold ----
            # x tiles 0..17: pass p, full k-tile k -> index p*6+k; tile 18
            # packs the three passes' 16-row remainders (rows 768..783 x3).
            with tc.tile_pool(name="fc1res", bufs=1) as fc1res:
                x3 = fc1res.tile([128, NXT, BC], BF16)
                w1 = fc1res.tile([128, K1, D1], FP8)
                # k-interleaved issue, n=0 halves first so PE starts early
                for k in range(K1F):
                    for p3 in range(3):
                        nc.sync.dma_start(
                            out=w1[:, k, p3 * 1024:(p3 + 1) * 1024],
                            in_=w1t[k][:, p3 * 1024:(p3 + 1) * 1024],
                        )
                    for p in range(NPASS):
                        nc.sync.dma_start(out=x3[:, p * K1F + k, 0:512],
                                          in_=x3t[p * K1F + k][:, 0:512])
                    if k == 0:
                        for p3 in range(3):
                            nc.sync.dma_start(
                                out=w1[:, K1F, p3 * 1024:(p3 + 1) * 1024],
                                in_=w1t[K1F][:, p3 * 1024:(p3 + 1) * 1024],
                            )
                        nc.sync.dma_start(out=x3[:, 18, 0:512],
                                          in_=x3t[18][:, 0:512])
                    if k == 1:
                        cv = consts.tile([128, NJ1 + 3 * NJ3], F32)
                        nc.sync.dma_start(out=cv, in_=cvec[:, :])
                        nt1 = cv[:, 0:NJ1]
                        nt2 = cv[:, NJ1:NJ1 + NJ3]
                        s3s = cv[:, NJ1 + NJ3:NJ1 + 2 * NJ3]
                        c3s = cv[:, NJ1 + 2 * NJ3:NJ1 + 3 * NJ3]
                        w4s = consts.tile([128, NJ3, NCLS], F32R)
                        nc.sync.dma_start(
                            out=w4s, in_=w4t.rearrange("p (j c) -> p j c", c=NCLS))
                        b4s = consts.tile([NCLS, 1], F32)
                        nc.sync.dma_start(out=b4s, in_=b4c[:, :])
                        for n in range(NB):
                            nc.vector.memset(lg[n], 0.0)
                for i in range(NXT):
                    nc.sync.dma_start(out=x3[:, i, 512:1024],
                                      in_=x3t[i][:, 512:1024])

                for n in range(NB):
                    for j in range(NJ1):
                        ps = psum.tile([128, 512], F32, tag="ps")
                        idx = 0
                        for k in range(K1F):
                            for p in range(NPASS):
                                nc.tensor.matmul(
                                    ps,
                                    lhsT=w1[:, k, j * 128:(j + 1) * 128],
                                    rhs=x3[:, p * K1F + k, n * 512:(n + 1) * 512],
                                    start=(idx == 0),
                                    stop=False,
                                )
                                idx += 1
                        nc.tensor.matmul(
                            ps,
                            lhsT=w1[:, K1F, j * 128:(j + 1) * 128],
                            rhs=x3[:, 18, n * 512:(n + 1) * 512],
                            start=False,
                            stop=True,
                        )
                        nc.scalar.activation(
                            out=a1[:, j // 2, j % 2, n * 512:(n + 1) * 512],
                            in_=ps,
                            func=ACTF.Sign,
                            bias=nt1[:, j:j + 1],
                            scale=1.0,
                        )

            # ---- fc2: fp8 DoubleRow + sign threshold ----
            with (
                tc.tile_pool(name="w3s", bufs=2) as w3s,
                tc.tile_pool(name="h3p", bufs=3) as h3p,
            ):
                for jb in range(NJ2 // JB):
                    wt = w2s.tile([128, NT2, 2, JB * 128], FP8, tag="w2t")
                    for t in range(NT2):
                        nc.sync.dma_start(out=wt[:, t], in_=w2p[jb, t])
                    for j in range(JB):
                        jj = jb * JB + j
                        for n in range(NB):
                            ps = psum.tile([128, 512], F32, tag="ps")
                            for t in range(NT2):
                                nc.tensor.matmul(
                                    ps,
                                    lhsT=wt[:, t, :, j * 128:(j + 1) * 128],
                                    rhs=a1[:, t, :, n * 512:(n + 1) * 512],
                                    start=(t == 0),
                                    stop=(t == NT2 - 1),
                                    perf_mode=DR,
                                )
                            nc.scalar.activation(
                                out=a2[:, jj // 2, jj % 2, n * 512:(n + 1) * 512],
                                in_=ps,
                                func=ACTF.Sign,
                                bias=nt2[:, jj:jj + 1],
                                scale=1.0,
                            )

                # ---- fc3 (fp8 DoubleRow) + bn3/hardtanh + fused fc4 ----
                for jb in range(NJ3 // JB):
                    wt = w3s.tile([128, NT3, 2, JB * 128], FP8, tag="w3t")
                    for tg in range(NT3 // 2):
                        nc.sync.dma_start(
                            out=wt[:, 2 * tg:2 * tg + 2],
                            in_=w3p[jb, 2 * tg:2 * tg + 2].rearrange(
                                "t p i n -> p t i n"),
                        )
                    for j in range(JB):
                        jj = jb * JB + j
                        h3 = h3p.tile([128, BC], F32R, tag="h3")
                        for n in range(NB):
                            ps = psum.tile([128, 512], F32, tag="ps")
                            for t in range(NT3):
                                nc.tensor.matmul(
                                    ps,
                                    lhsT=wt[:, t, :, j * 128:(j + 1) * 128],
                                    rhs=a2[:, t, :, n * 512:(n + 1) * 512],
                                    start=(t == 0),
                                    stop=(t == NT3 - 1),
                                    perf_mode=DR,
                                )
                            tmp = h3p.tile([128, 512], F32, tag="bn3tmp")
                            nc.scalar.activation(
                                out=tmp,
                                in_=ps,
                                func=ACTF.Identity,
                                bias=c3s[:, jj:jj + 1],
                                scale=s3s[:, jj:jj + 1],
                            )
                            nc.vector.tensor_scalar(
                                out=h3[:, n * 512:(n + 1) * 512],
                                in0=tmp,
                                scalar1=-1.0,
                                scalar2=1.0,
                                op0=ALU.max,
                                op1=ALU.min,
                            )
                            # fused fc4 (fp32r): lg[n][c, b] += w4[c,:] @ h3c[:, b]
                            nc.tensor.matmul(
                                lg[n],
                                lhsT=w4s[:, jj, :],
                                rhs=h3[:, n * 512:(n + 1) * 512],
                                start=False,
                                stop=(jj == NJ3 - 1),
                                skip_group_check=True,
                            )

            # ---- epilogue: +b4, transpose [cls,b]->[b,cls], log_softmax ----
            with tc.tile_pool(name="epi", bufs=2) as epi:
                ident = consts.tile([NCLS, NCLS], F32)
                make_identity(nc, ident)
                lsb = epi.tile([NCLS, BC], F32, tag="lsb")
                for n in range(NB):
                    nc.scalar.activation(
                        out=lsb[:, n * 512:(n + 1) * 512],
                        in_=lg[n],
                        func=ACTF.Identity,
                        bias=b4s[:, 0:1],
                        scale=1.0,
                    )
                tp = psum_tp.tile([128, NBCH, NCLS], F32)
                for b in range(NBCH):
                    nc.tensor.transpose(
                        tp[:, b, :], lsb[:, b * 128:(b + 1) * 128], ident)
                # log_softmax without max-shift: logits are O(5), exp safe
                ex = epi.tile([128, NBCH, 10], F32, tag="ex")
                nc.scalar.activation(out=ex, in_=tp[:, :, 0:10], func=ACTF.Exp)
                sm = epi.tile([128, NBCH], F32, tag="sm")
                nc.vector.tensor_reduce(
                    out=sm, in_=ex, axis=mybir.AxisListType.X, op=ALU.add)
                lnt = epi.tile([128, NBCH], F32, tag="lnt")
                nc.scalar.activation(out=lnt, in_=sm, func=ACTF.Ln)
                for b in range(NBCH):
                    res = epi.tile([128, 10], F32, tag="res")
                    nc.vector.tensor_scalar(
                        out=res, in0=tp[:, b, 0:10], scalar1=lnt[:, b:b + 1],
                        scalar2=None, op0=ALU.subtract,
                    )
                    nc.sync.dma_start(out=out[b * 128:(b + 1) * 128, :], in_=res)

    _legalize_multiwait(nc)
    return nc


def _split3(x):
    """x (fp32) -> three bf16 arrays summing to x with <=2^-25 rel error."""
    x1 = x.astype(NP_BF16)
    r1 = (x - x1.astype(np.float32)).astype(np.float32)
    x2 = r1.astype(NP_BF16)
    r2 = (r1 - x2.astype(np.float32)).astype(np.float32)
    x3 = r2.astype(NP_BF16)
    return x1, x2, x3


def _prep_inputs(inputs):
    f64 = {k: np.asarray(v, np.float64) for k, v in inputs.items()
           if k != "x"}
    x = np.asarray(inputs["x"], np.float32)

    s1 = f64["g1"] / np.sqrt(f64["v1"] + EPS)
    t1 = f64["m1"] - f64["b1"] - f64["be1"] / s1
    s2 = f64["g2"] / np.sqrt(f64["v2"] + EPS)
    t2 = f64["m2"] - f64["b2"] - f64["be2"] / s2
    s3 = f64["g3"] / np.sqrt(f64["v3"] + EPS)
    c3 = (f64["b3"] - f64["m3"]) * s3 + f64["be3"]

    shared = {}
    # cvec [128, 24+48*3]: per-feature consts arranged [partition, tile]
    cvec = np.zeros((128, NJ1 + 3 * NJ3), np.float32)
    cvec[:, 0:NJ1] = (-t1).astype(np.float32).reshape(NJ1, 128).T
    cvec[:, NJ1:NJ1 + NJ3] = (-t2).astype(np.float32).reshape(NJ3, 128).T
    cvec[:, NJ1 + NJ3:NJ1 + 2 * NJ3] = s3.astype(np.float32).reshape(NJ3, 128).T
    cvec[:, NJ1 + 2 * NJ3:] = c3.astype(np.float32).reshape(NJ3, 128).T
    shared["cvec"] = np.ascontiguousarray(cvec)

    b4p = np.zeros((NCLS, 1), np.float32)
    b4p[:10, 0] = np.asarray(inputs["b4"], np.float32)
    shared["b4c"] = b4p

    # w1: sign, transposed to [in, out]; slots 0..5 = rows 0..767,
    # slot 6 = the 16 remainder rows replicated 3x (one per x pass) + zeros
    w1b = np.sign(np.asarray(inputs["w1"], np.float32)).astype(np.float32)
    w1T = w1b.T  # [784, D1]
    w1arr = np.zeros((K1, 128, D1), np.float32)
    w1arr[:K1F] = w1T[:768].reshape(K1F, 128, D1)
    for p in range(3):
        w1arr[K1F, 16 * p:16 * (p + 1)] = w1T[768:784]
    shared["w1t"] = np.ascontiguousarray(w1arr.astype(NP_FP8))

    # w2/w3: sign -> DoubleRow pair layout [njb, nt, 128, 2, JB*128] fp8
    def pack_dr(w, njb_out):
        wT = np.sign(np.asarray(w, np.float32)).T  # [in, out]
        nin, nout = wT.shape
        nt = nin // 256
        a = wT.reshape(nt, 2, 128, nout).transpose(0, 2, 1, 3)  # [nt,128,2,out]
        a = a.reshape(nt, 128, 2, njb_out, JB * 128).transpose(3, 0, 1, 2, 4)
        return np.ascontiguousarray(a.astype(NP_FP8))

    shared["w2p"] = pack_dr(inputs["w2"], NJ2 // JB)
    shared["w3p"] = pack_dr(inputs["w3"], NJ3 // JB)

    # w4: [10, D2] -> [128, NJ3*NCLS]: element [k, j*16+c] = w4[c, j*128+k]
    w4 = np.asarray(inputs["w4"], np.float32)
    w4tp = np.zeros((D2, NCLS), np.float32)
    w4tp[:, :10] = w4.T
    shared["w4t"] = np.ascontiguousarray(
        w4tp.reshape(NJ3, 128, NCLS).transpose(1, 0, 2).reshape(128, NJ3 * NCLS))

    # x: transpose, split into 3 bf16 passes; tiles 0..17 = [pass, full-k],
    # tile 18 packs the three passes' remainder rows 768..783 (+ zero pad)
    xT = np.ascontiguousarray(x.T)  # [784, B]
    splits = _split3(xT)
    per_core = []
    for c in range(NCORES):
        sl = slice(c * BC, (c + 1) * BC)
        x3t = np.zeros((NXT, 128, BC), NP_BF16)
        for p, xi in enumerate(splits):
            x3t[p * K1F:(p + 1) * K1F] = xi[:768, sl].reshape(K1F, 128, BC)
            x3t[18, 16 * p:16 * (p + 1)] = xi[768:784, sl]
        m = dict(shared)
        m["x3t"] = np.ascontiguousarray(x3t)
        per_core.append(m)
    return per_core


_NC_CACHE = None


def kernel(**inputs):
    global _NC_CACHE, LAST_EXEC_NS
    if _NC_CACHE is None:
        _NC_CACHE = _build_nc()
    nc = _NC_CACHE
    in_maps = _prep_inputs(inputs)
    kwargs = {}
    if TRACE:
        _install_ntff_shim()
        kwargs = dict(trace=True, tmpdir=TRACE_DIR)
    res = None
    outs = None
    for attempt in range(3):
        try:
            res = run_bass_kernel_spmd(nc, in_maps, core_ids=list(range(NCORES)),
                                       **kwargs)
            outs = [np.asarray(res.results[c]["out"]) for c in range(NCORES)]
            break
        except Exception:
            if attempt == 2:
                raise
    LAST_EXEC_NS = res.exec_time_ns
    return np.concatenate(outs, axis=0)


def _install_ntff_shim():
    """antenv.axon_hooks shim so trace=True works under axon (profiling only)."""
    import contextlib
    import ctypes
    import types

    if "antenv.axon_hooks" in sys.modules:
        return
    try:
        lib = ctypes.CDLL("/opt/axon/libaxon_pjrt.so")
        lib.axon_start_nrt_profile.argtypes = [
            ctypes.POINTER(ctypes.c_int64), ctypes.c_size_t]
        lib.axon_start_nrt_profile.restype = ctypes.c_int64
        lib.axon_stop_nrt_profile.argtypes = [ctypes.c_char_p]
        lib.axon_stop_nrt_profile.restype = ctypes.c_int64
    except (OSError, AttributeError):
        return

    @contextlib.contextmanager
    def _hook(output_dir, device_ids):
        import jax
        jax.devices()
        if device_ids:
            ids = (ctypes.c_int64 * len(device_ids))(*device_ids)
            rc = lib.axon_start_nrt_profile(ids, len(device_ids))
        else:
            rc = lib.axon_start_nrt_profile(None, 0)
        if rc != 0:
            raise RuntimeError(f"axon_start_nrt_profile rc={rc}")
        try:
            yield
        finally:
            n = lib.axon_stop_nrt_profile(str(output_dir).encode())
            print(f"ntff: {n} profile file(s) -> {output_dir}", file=sys.stderr)

    mod = types.ModuleType("antenv.axon_hooks")
    mod.get_axon_ntff_profile_hook = lambda: _hook
    mod.set_axon_ntff_profile_hook = lambda h: None
    sys.modules["antenv.axon_hooks"] = mod



# revision 6
# speedup vs baseline: 1.2035x; 1.0313x over previous
"""Trainium2 Bass kernel for the Binary-MLP (nn_Binary0) problem.

Strategy (8-way batch-parallel, 1024 rows/core):
  fc1: h1 = x @ sign(w1).T        -- fp16x2 split of x (exact to ~2^-22):
       pass1 rhs = fp16(x), lhsT = +-1; pass2 rhs = fp16((x-x1)*2^11),
       lhsT = +-2^-11 (both weight scales exact in fp8e5m2). 13 k-tiles
       vs 19 for the old bf16x3 split. k-outer loop over j-groups of 3
       so the PE saturates as soon as the first x k-tiles land.
       a1 = sign(h1 - t1)          -- thresholds fold bias+BN (host fp64)
  fc2: h2 = a1 @ sign(w2).T        -- fp8 DoubleRow (exact: +-1 products)
       a2 = sign(h2 - t2)
  fc3: h3 = a2 @ sign(w3).T        -- fp8 DoubleRow
       h3c = clip(h3*s3 + c3, -1, 1) -> fp16
  fc4: logits.T = w4 @ h3c         -- fp16 (216ns/MM vs 380 for f32r),
                                      fused into fc3 loop, [cls, batch]
  out = log_softmax(logits)        -- PE-transpose, free-dim reduce,
                                      single batched output DMA

DMA: triggers cost ~650ns serially per queue (~200GB/s per queue for
128KB transfers) -> few big chunked DMAs, split across the sync and
scalar (Activation) hardware queues at startup.
"""
import sys

for _p in ("/opt/trn_rl_repo",):
    if _p not in sys.path:
        sys.path.insert(0, _p)

import numpy as np
import ml_dtypes

import concourse.bass as bass
import concourse.tile as tile
import concourse.mybir as mybir
from concourse.bass_utils import run_bass_kernel_spmd
from concourse.masks import make_identity

F32 = mybir.dt.float32
F16 = mybir.dt.float16
BF16 = mybir.dt.bfloat16
FP8 = mybir.dt.float8e4
FP8E5 = mybir.dt.float8e5
NP_FP8 = mybir.dt.np(FP8)
NP_FP8E5 = mybir.dt.np(FP8E5)

EPS = 1e-5
NCORES = 8
B = 8192
BC = B // NCORES            # 1024 batch rows per core
D0, D1, D2 = 784, 3072, 6144
K1 = 13                     # fc1 k-tiles: 6 pass1 + 6 pass2 + 1 packed
K1F = 6                     # full 128-row k-tiles per pass (768 rows)
NJ1 = D1 // 128             # 24 fc1 output feature tiles
G1 = 3                      # fc1 j-tiles per psum group
NG1 = NJ1 // G1             # 8 groups
NT2 = D1 // 256             # 12 fc2 DoubleRow contraction tiles
NJ2 = D2 // 128             # 48
NT3 = D2 // 256             # 24 fc3 DoubleRow contraction tiles
NJ3 = D2 // 128             # 48
JB = 4                      # j-tiles per streamed weight slab
NB = 2                      # 512-wide batch halves of BC
NBCH = BC // 128            # 8 batch chunks
NCLS = 16                   # padded class dim (10 real)
S2L = 2.0 ** 11             # pass2 rhs scale
S2W = 2.0 ** -11            # pass2 weight scale

TRACE = False               # test.py sets True for profiling
TRACE_DIR = None
LAST_EXEC_NS = None

DR = mybir.MatmulPerfMode.DoubleRow
ACTF = mybir.ActivationFunctionType
ALU = mybir.AluOpType


def _legalize_multiwait(nc):
    """This container's walrus build rejects >1 sync-wait on one instruction
    (codegen 'Too many sync wait commands'); split extra waits into NoOps."""
    n = 0
    for f in nc.m.functions:
        for blk in f.blocks:
            insts = list(blk.instructions)
            new = []
            changed = False
            for ins in insts:
                si = ins.sync_info
                waits = list(si.on_wait) if (si is not None and si.on_wait) else []
                if len(waits) > 1:
                    for k, w in enumerate(waits[:-1]):
                        nop = mybir.InstNoOp(name=f"{ins.name}-sw{k}", ins=[], outs=[])
                        nop.engine = ins.engine
                        nop.sync_info = mybir.SyncInfo(on_wait=[w], on_update=[])
                        new.append(nop)
                        n += 1
                    ins.sync_info = mybir.SyncInfo(
                        on_wait=[waits[-1]], on_update=list(si.on_update or [])
                    )
                    changed = True
                new.append(ins)
            if changed:
                blk.instructions = new
    return n


def _build_nc():
    nc = bass.Bass("TRN2")

    xht = nc.dram_tensor("xht", [128, K1 * BC], F16, kind="ExternalInput")
    w1t = nc.dram_tensor("w1t", [128, NG1 * K1 * G1 * 128], FP8E5,
                         kind="ExternalInput")
    w2p = nc.dram_tensor("w2p", [NJ2 // JB, 128, NT2 * 2 * JB * 128], FP8,
                         kind="ExternalInput")
    w3p = nc.dram_tensor("w3p", [NJ3 // JB, 128, NT3 * 2 * JB * 128], FP8,
                         kind="ExternalInput")
    w4t = nc.dram_tensor("w4t", [128, NJ3 * NCLS], F16, kind="ExternalInput")
    # cvec columns: [0:24]=-t1, [24:72]=-t2, [72:120]=s3, [120:168]=c3
    cvec = nc.dram_tensor("cvec", [128, NJ1 + 3 * NJ3], F32, kind="ExternalInput")
    b4c = nc.dram_tensor("b4c", [NCLS, 1], F32, kind="ExternalInput")
    out = nc.dram_tensor("out", [BC, 10], F32, kind="ExternalOutput")

    xr = xht.rearrange("p (k c) -> p k c", c=BC)
    wr = w1t.rearrange("p (g k c) -> p g k c", k=K1, c=G1 * 128)

    with tile.TileContext(nc) as tc:
        with (
            tc.tile_pool(name="consts", bufs=1) as consts,
            tc.tile_pool(name="a1p", bufs=1) as a1p,
            tc.tile_pool(name="a2p", bufs=1) as a2p,
            tc.tile_pool(name="psum", bufs=5, space="PSUM") as psum,
            tc.tile_pool(name="psum_lg", bufs=2, space="PSUM") as psum_lg,
            tc.tile_pool(name="psum_tp", bufs=1, space="PSUM") as psum_tp,
        ):
            a1 = a1p.tile([128, NT2, 2, BC], FP8)
            a2 = a2p.tile([128, NT3, 2, BC], FP8)

            # fc4 logits accumulators [cls, batch-half], pre-zeroed, start=False
            lg = [psum_lg.tile([NCLS, 512], F32, tag="lg", name=f"lg{i}")
                  for i in range(NB)]

            # ---- fc1: fp16x2 exact split + sign threshold ----
            with tc.tile_pool(name="fc1res", bufs=1) as fc1res:
                xh = fc1res.tile([128, K1, BC], F16)
                w1s = fc1res.tile([128, NG1, K1, G1 * 128], FP8E5)

                # startup-critical DMAs, split across the sync + scalar
                # hardware queues; x n=0 chunks pace the first j-group.
                nc.sync.dma_start(out=xh[:, 0:3, 0:512], in_=xr[:, 0:3, 0:512])
                nc.sync.dma_start(out=xh[:, 3:6, 0:512], in_=xr[:, 3:6, 0:512])
                nc.sync.dma_start(out=xh[:, 6:9, 0:512], in_=xr[:, 6:9, 0:512])
                nc.sync.dma_start(out=xh[:, 9:K1, 0:512], in_=xr[:, 9:K1, 0:512])
                for g in range(NG1):
                    nc.scalar.dma_start(out=w1s[:, g], in_=wr[:, g])
                cv = consts.tile([128, NJ1 + 3 * NJ3], F32)
                nc.sync.dma_start(out=cv, in_=cvec[:, :])
                nt1 = cv[:, 0:NJ1]
                nt2 = cv[:, NJ1:NJ1 + NJ3]
                s3s = cv[:, NJ1 + NJ3:NJ1 + 2 * NJ3]
                c3s = cv[:, NJ1 + 2 * NJ3:NJ1 + 3 * NJ3]
                w4s = consts.tile([128, NJ3, NCLS], F16)
                nc.sync.dma_start(
                    out=w4s, in_=w4t.rearrange("p (j c) -> p j c", c=NCLS))
                b4s = consts.tile([NCLS, 1], F32)
                nc.sync.dma_start(out=b4s, in_=b4c[:, :])
                # x n=1 half: needed only after phase 0 (~65us in)
                nc.sync.dma_start(out=xh[:, 0:7, 512:1024],
                                  in_=xr[:, 0:7, 512:1024])
                nc.sync.dma_start(out=xh[:, 7:K1, 512:1024],
                                  in_=xr[:, 7:K1, 512:1024])
                for n in range(NB):
                    nc.vector.memset(lg[n], 0.0)

                for n in range(NB):
                    for g in range(NG1):
                        pss = [psum.tile([128, 512], F32, tag="ps",
                                         name=f"f1_{n}_{g}_{i}")
                               for i in range(G1)]
                        for k in range(K1):
                            for j3 in range(G1):
                                nc.tensor.matmul(
                                    pss[j3],
                                    lhsT=w1s[:, g, k, j3 * 128:(j3 + 1) * 128],
                                    rhs=xh[:, k, n * 512:(n + 1) * 512],
                                    start=(k == 0),
                                    stop=(k == K1 - 1),
                                )
                        for j3 in range(G1):
                            j = g * G1 + j3
                            nc.scalar.activation(
                                out=a1[:, j // 2, j % 2, n * 512:(n + 1) * 512],
                                in_=pss[j3],
                                func=ACTF.Sign,
                                bias=nt1[:, j:j + 1],
                                scale=1.0,
                            )

            # ---- fc2: fp8 DoubleRow + sign threshold ----
            with tc.tile_pool(name="w2s", bufs=2) as w2s:
                for jb in range(NJ2 // JB):
                    wt = w2s.tile([128, NT2, 2, JB * 128], FP8, tag="w2t")
                    w2r = w2p[jb].rearrange("p (t i c) -> p t i c",
                                            i=2, c=JB * 128)
                    for tg in range(NT2 // 3):
                        nc.sync.dma_start(out=wt[:, 3 * tg:3 * tg + 3],
                                          in_=w2r[:, 3 * tg:3 * tg + 3])
                    for j in range(JB):
                        jj = jb * JB + j
                        for n in range(NB):
                            ps = psum.tile([128, 512], F32, tag="ps")
                            for t in range(NT2):
                                nc.tensor.matmul(
                                    ps,
                                    lhsT=wt[:, t, :, j * 128:(j + 1) * 128],
                                    rhs=a1[:, t, :, n * 512:(n + 1) * 512],
                                    start=(t == 0),
                                    stop=(t == NT2 - 1),
                                    perf_mode=DR,
                                )
                            nc.scalar.activation(
                                out=a2[:, jj // 2, jj % 2, n * 512:(n + 1) * 512],
                                in_=ps,
                                func=ACTF.Sign,
                                bias=nt2[:, jj:jj + 1],
                                scale=1.0,
                            )

            # ---- fc3 (fp8 DoubleRow) + bn3/hardtanh + fused fc4 (fp16) ----
            with (
                tc.tile_pool(name="w3s", bufs=2) as w3s,
                tc.tile_pool(name="h3p", bufs=3) as h3p,
            ):
                for jb in range(NJ3 // JB):
                    wt = w3s.tile([128, NT3, 2, JB * 128], FP8, tag="w3t")
                    w3r = w3p[jb].rearrange("p (t i c) -> p t i c",
                                            i=2, c=JB * 128)
                    for tg in range(NT3 // 4):
                        nc.sync.dma_start(out=wt[:, 4 * tg:4 * tg + 4],
                                          in_=w3r[:, 4 * tg:4 * tg + 4])
                    for j in range(JB):
                        jj = jb * JB + j
                        h3 = h3p.tile([128, BC], F16, tag="h3")
                        for n in range(NB):
                            ps = psum.tile([128, 512], F32, tag="ps")
                            for t in range(NT3):
                                nc.tensor.matmul(
                                    ps,
                                    lhsT=wt[:, t, :, j * 128:(j + 1) * 128],
                                    rhs=a2[:, t, :, n * 512:(n + 1) * 512],
                                    start=(t == 0),
                                    stop=(t == NT3 - 1),
                                    perf_mode=DR,
                                )
                            tmp = h3p.tile([128, 512], F32, tag="bn3tmp")
                            nc.scalar.activation(
                                out=tmp,
                                in_=ps,
                                func=ACTF.Identity,
                                bias=c3s[:, jj:jj + 1],
                                scale=s3s[:, jj:jj + 1],
                            )
                            nc.vector.tensor_scalar(
                                out=h3[:, n * 512:(n + 1) * 512],
                                in0=tmp,
                                scalar1=-1.0,
                                scalar2=1.0,
                                op0=ALU.max,
                                op1=ALU.min,
                            )
                            # fused fc4 (fp16): lg[n][c, b] += w4[c,:] @ h3c[:, b]
                            nc.tensor.matmul(
                                lg[n],
                                lhsT=w4s[:, jj, :],
                                rhs=h3[:, n * 512:(n + 1) * 512],
                                start=False,
                                stop=(jj == NJ3 - 1),
                                skip_group_check=True,
                            )

            # ---- epilogue: +b4, transpose [cls,b]->[b,cls], log_softmax ----
            with tc.tile_pool(name="epi", bufs=1) as epi:
                ident = consts.tile([NCLS, NCLS], F32)
                make_identity(nc, ident)
                lsb = epi.tile([NCLS, BC], F32, tag="lsb")
                tp = psum_tp.tile([128, NBCH, NCLS], F32, tag="tp")
                for n in range(NB):
                    nc.scalar.activation(
                        out=lsb[:, n * 512:(n + 1) * 512],
                        in_=lg[n],
                        func=ACTF.Identity,
                        bias=b4s[:, 0:1],
                        scale=1.0,
                    )
                    for b in range(4):
                        c = n * 4 + b
                        nc.tensor.transpose(
                            tp[:, c, :], lsb[:, c * 128:(c + 1) * 128], ident)
                # log_softmax without max-shift: logits are O(5), exp safe
                ex = epi.tile([128, NBCH, 10], F32, tag="ex")
                nc.scalar.activation(out=ex, in_=tp[:, :, 0:10], func=ACTF.Exp)
                sm = epi.tile([128, NBCH], F32, tag="sm")
                nc.vector.tensor_reduce(
                    out=sm, in_=ex, axis=mybir.AxisListType.X, op=ALU.add)
                lnt = epi.tile([128, NBCH], F32, tag="lnt")
                nc.scalar.activation(out=lnt, in_=sm, func=ACTF.Ln)
                res = epi.tile([128, NBCH, 10], F32, tag="res")
                for b in range(NBCH):
                    nc.vector.tensor_scalar(
                        out=res[:, b, :], in0=tp[:, b, 0:10],
                        scalar1=lnt[:, b:b + 1],
                        scalar2=None, op0=ALU.subtract,
                    )
                nc.sync.dma_start(
                    out=out.rearrange("(c p) f -> p c f", p=128), in_=res)

    _legalize_multiwait(nc)
    return nc


def _prep_inputs(inputs):
    f64 = {k: np.asarray(v, np.float64) for k, v in inputs.items()
           if k != "x"}
    x = np.asarray(inputs["x"], np.float32)

    s1 = f64["g1"] / np.sqrt(f64["v1"] + EPS)
    t1 = f64["m1"] - f64["b1"] - f64["be1"] / s1
    s2 = f64["g2"] / np.sqrt(f64["v2"] + EPS)
    t2 = f64["m2"] - f64["b2"] - f64["be2"] / s2
    s3 = f64["g3"] / np.sqrt(f64["v3"] + EPS)
    c3 = (f64["b3"] - f64["m3"]) * s3 + f64["be3"]

    shared = {}
    # cvec [128, 24+48*3]: per-feature consts arranged [partition, tile]
    cvec = np.zeros((128, NJ1 + 3 * NJ3), np.float32)
    cvec[:, 0:NJ1] = (-t1).astype(np.float32).reshape(NJ1, 128).T
    cvec[:, NJ1:NJ1 + NJ3] = (-t2).astype(np.float32).reshape(NJ3, 128).T
    cvec[:, NJ1 + NJ3:NJ1 + 2 * NJ3] = s3.astype(np.float32).reshape(NJ3, 128).T
    cvec[:, NJ1 + 2 * NJ3:] = c3.astype(np.float32).reshape(NJ3, 128).T
    shared["cvec"] = np.ascontiguousarray(cvec)

    b4p = np.zeros((NCLS, 1), np.float32)
    b4p[:10, 0] = np.asarray(inputs["b4"], np.float32)
    shared["b4c"] = b4p

    # w1: sign, transposed to [in, out]; k-tiles 0-5 = pass1 rows (+-1),
    # 6-11 = pass2 rows (+-2^-11), 12 = packed remainders of both passes.
    # Then permuted to j-group-major so each group is one contiguous DMA.
    w1b = np.sign(np.asarray(inputs["w1"], np.float32)).astype(np.float32)
    w1T = w1b.T  # [784, D1]
    w1f = np.zeros((128, K1, D1), np.float32)
    for k in range(K1F):
        w1f[:, k, :] = w1T[k * 128:(k + 1) * 128]
        w1f[:, k + K1F, :] = w1T[k * 128:(k + 1) * 128] * S2W
    w1f[0:16, 12, :] = w1T[768:784]
    w1f[16:32, 12, :] = w1T[768:784] * S2W
    w1e5 = w1f.astype(NP_FP8E5)
    shared["w1t"] = np.ascontiguousarray(
        w1e5.reshape(128, K1, NG1, G1 * 128).transpose(0, 2, 1, 3)
        .reshape(128, NG1 * K1 * G1 * 128))

    # w2/w3: sign -> DoubleRow pair layout, slab-contiguous per partition:
    # [njb, 128, nt*2*(JB*128)] fp8
    def pack_dr(w, njb_out):
        wT = np.sign(np.asarray(w, np.float32)).T  # [in, out]
        nin, nout = wT.shape
        nt = nin // 256
        a = wT.reshape(nt, 2, 128, nout).transpose(0, 2, 1, 3)  # [nt,128,2,out]
        a = a.reshape(nt, 128, 2, njb_out, JB * 128).transpose(3, 1, 0, 2, 4)
        # a: [njb, 128, nt, 2, JB*128]
        return np.ascontiguousarray(
            a.reshape(njb_out, 128, nt * 2 * JB * 128).astype(NP_FP8))

    shared["w2p"] = pack_dr(inputs["w2"], NJ2 // JB)
    shared["w3p"] = pack_dr(inputs["w3"], NJ3 // JB)

    # w4: [10, D2] -> fp16 [128, NJ3*NCLS]: elem [k, j*16+c] = w4[c, j*128+k]
    w4 = np.asarray(inputs["w4"], np.float32)
    w4tp = np.zeros((D2, NCLS), np.float32)
    w4tp[:, :10] = w4.T
    shared["w4t"] = np.ascontiguousarray(
        w4tp.reshape(NJ3, 128, NCLS).transpose(1, 0, 2)
        .reshape(128, NJ3 * NCLS).astype(np.float16))

    # x: transpose, fp16x2 split (pass2 scaled by 2^11); per-core layout
    # [128, K1*BC] with k-tile-major columns.
    xT = np.ascontiguousarray(x.T)  # [784, B]
    x1 = xT.astype(np.float16)
    x2s = ((xT - x1.astype(np.float32)) * S2L).astype(np.float16)
    per_core = []
    for cix in range(NCORES):
        sl = slice(cix * BC, (cix + 1) * BC)
        xa = np.zeros((K1, 128, BC), np.float16)
        xa[0:K1F] = x1[0:768, sl].reshape(K1F, 128, BC)
        xa[K1F:2 * K1F] = x2s[0:768, sl].reshape(K1F, 128, BC)
        xa[12, 0:16] = x1[768:784, sl]
        xa[12, 16:32] = x2s[768:784, sl]
        m = dict(shared)
        m["xht"] = np.ascontiguousarray(
            xa.transpose(1, 0, 2).reshape(128, K1 * BC))
        per_core.append(m)
    return per_core


_NC_CACHE = None


def kernel(**inputs):
    global _NC_CACHE, LAST_EXEC_NS
    if _NC_CACHE is None:
        _NC_CACHE = _build_nc()
    nc = _NC_CACHE
    in_maps = _prep_inputs(inputs)
    kwargs = {}
    if TRACE:
        _install_ntff_shim()
        kwargs = dict(trace=True, tmpdir=TRACE_DIR)
    res = None
    outs = None
    for attempt in range(3):
        try:
            res = run_bass_kernel_spmd(nc, in_maps, core_ids=list(range(NCORES)),
                                       **kwargs)
            outs = [np.asarray(res.results[c]["out"]) for c in range(NCORES)]
            break
        except Exception:
            if attempt == 2:
                raise
    LAST_EXEC_NS = res.exec_time_ns
    return np.concatenate(outs, axis=0)


def _install_ntff_shim():
    """antenv.axon_hooks shim so trace=True works under axon (profiling only)."""
    import contextlib
    import ctypes
    import types

    if "antenv.axon_hooks" in sys.modules:
        return
    try:
        lib = ctypes.CDLL("/opt/axon/libaxon_pjrt.so")
        lib.axon_start_nrt_profile.argtypes = [
            ctypes.POINTER(ctypes.c_int64), ctypes.c_size_t]
        lib.axon_start_nrt_profile.restype = ctypes.c_int64
        lib.axon_stop_nrt_profile.argtypes = [ctypes.c_char_p]
        lib.axon_stop_nrt_profile.restype = ctypes.c_int64
    except (OSError, AttributeError):
        return

    @contextlib.contextmanager
    def _hook(output_dir, device_ids):
        import jax
        jax.devices()
        if device_ids:
            ids = (ctypes.c_int64 * len(device_ids))(*device_ids)
            rc = lib.axon_start_nrt_profile(ids, len(device_ids))
        else:
            rc = lib.axon_start_nrt_profile(None, 0)
        if rc != 0:
            raise RuntimeError(f"axon_start_nrt_profile rc={rc}")
        try:
            yield
        finally:
            n = lib.axon_stop_nrt_profile(str(output_dir).encode())
            print(f"ntff: {n} profile file(s) -> {output_dir}", file=sys.stderr)

    mod = types.ModuleType("antenv.axon_hooks")
    mod.get_axon_ntff_profile_hook = lambda: _hook
    mod.set_axon_ntff_profile_hook = lambda h: None
    sys.modules["antenv.axon_hooks"] = mod


# revision 7
# speedup vs baseline: 1.2266x; 1.0192x over previous
"""Trainium2 Bass kernel for the Binary-MLP (nn_Binary0) problem.

Strategy (8-way batch-parallel, 1024 rows/core):
  fc1: h1 = x @ sign(w1).T        -- fp16x2 split of x (exact to ~2^-22):
       pass1 rhs = fp16(x), lhsT = +-1; pass2 rhs = fp16((x-x1)*2^11),
       lhsT = +-2^-11 (both weight scales exact in fp8e5m2). 13 k-tiles
       vs 19 for the old bf16x3 split. k-outer loop over j-groups of 3
       so the PE saturates as soon as the first x k-tiles land.
       a1 = sign(h1 - t1)          -- thresholds fold bias+BN (host fp64)
  fc2: h2 = a1 @ sign(w2).T        -- fp8 DoubleRow (exact: +-1 products)
       a2 = sign(h2 - t2)
  fc3: h3 = a2 @ sign(w3).T        -- fp8 DoubleRow
       h3c = clip(h3*s3 + c3, -1, 1) -> fp16
  fc4: logits.T = w4 @ h3c         -- fp16 (216ns/MM vs 380 for f32r),
                                      fused into fc3 loop, [cls, batch]
  out = log_softmax(logits)        -- PE-transpose, free-dim reduce,
                                      single batched output DMA

DMA: triggers cost ~650ns serially per queue (~200GB/s per queue for
128KB transfers) -> few big chunked DMAs, split across the sync and
scalar (Activation) hardware queues at startup.
"""
import sys

for _p in ("/opt/trn_rl_repo",):
    if _p not in sys.path:
        sys.path.insert(0, _p)

import numpy as np
import ml_dtypes

import concourse.bass as bass
import concourse.tile as tile
import concourse.mybir as mybir
from concourse.bass_utils import run_bass_kernel_spmd
from concourse.masks import make_identity

F32 = mybir.dt.float32
F16 = mybir.dt.float16
BF16 = mybir.dt.bfloat16
FP8 = mybir.dt.float8e4
FP8E5 = mybir.dt.float8e5
NP_FP8 = mybir.dt.np(FP8)
NP_FP8E5 = mybir.dt.np(FP8E5)

EPS = 1e-5
NCORES = 8
B = 8192
BC = B // NCORES            # 1024 batch rows per core
D0, D1, D2 = 784, 3072, 6144
K1 = 13                     # fc1 k-tiles: 6 pass1 + 6 pass2 + 1 packed
K1F = 6                     # full 128-row k-tiles per pass (768 rows)
NJ1 = D1 // 128             # 24 fc1 output feature tiles
G1 = 3                      # fc1 j-tiles per psum group
NG1 = NJ1 // G1             # 8 groups
NT2 = D1 // 256             # 12 fc2 DoubleRow contraction tiles
NJ2 = D2 // 128             # 48
NT3 = D2 // 256             # 24 fc3 DoubleRow contraction tiles
NJ3 = D2 // 128             # 48
JB = 4                      # j-tiles per streamed weight slab
NB = 2                      # 512-wide batch halves of BC
NBCH = BC // 128            # 8 batch chunks
NCLS = 16                   # padded class dim (10 real)
S2L = 2.0 ** 11             # pass2 rhs scale
S2W = 2.0 ** -11            # pass2 weight scale

TRACE = False               # test.py sets True for profiling
TRACE_DIR = None
LAST_EXEC_NS = None

DR = mybir.MatmulPerfMode.DoubleRow
ACTF = mybir.ActivationFunctionType
ALU = mybir.AluOpType


def _legalize_multiwait(nc):
    """This container's walrus build rejects >1 sync-wait on one instruction
    (codegen 'Too many sync wait commands'); split extra waits into NoOps."""
    n = 0
    for f in nc.m.functions:
        for blk in f.blocks:
            insts = list(blk.instructions)
            new = []
            changed = False
            for ins in insts:
                si = ins.sync_info
                waits = list(si.on_wait) if (si is not None and si.on_wait) else []
                if len(waits) > 1:
                    for k, w in enumerate(waits[:-1]):
                        nop = mybir.InstNoOp(name=f"{ins.name}-sw{k}", ins=[], outs=[])
                        nop.engine = ins.engine
                        nop.sync_info = mybir.SyncInfo(on_wait=[w], on_update=[])
                        new.append(nop)
                        n += 1
                    ins.sync_info = mybir.SyncInfo(
                        on_wait=[waits[-1]], on_update=list(si.on_update or [])
                    )
                    changed = True
                new.append(ins)
            if changed:
                blk.instructions = new
    return n


def _build_nc():
    nc = bass.Bass("TRN2")

    xht = nc.dram_tensor("xht", [128, K1 * BC], F16, kind="ExternalInput")
    w1t = nc.dram_tensor("w1t", [128, NG1 * K1 * G1 * 128], FP8E5,
                         kind="ExternalInput")
    w2p = nc.dram_tensor("w2p", [NJ2 // JB, 128, NT2 * 2 * JB * 128], FP8,
                         kind="ExternalInput")
    w3p = nc.dram_tensor("w3p", [NJ3 // JB, 128, NT3 * 2 * JB * 128], FP8,
                         kind="ExternalInput")
    w4t = nc.dram_tensor("w4t", [128, NJ3 * NCLS], F16, kind="ExternalInput")
    # cvec columns: [0:24]=-t1, [24:72]=-t2, [72:120]=s3, [120:168]=c3
    cvec = nc.dram_tensor("cvec", [128, NJ1 + 3 * NJ3], F32, kind="ExternalInput")
    b4c = nc.dram_tensor("b4c", [NCLS, 1], F32, kind="ExternalInput")
    out = nc.dram_tensor("out", [BC, 10], F32, kind="ExternalOutput")

    xr = xht.rearrange("p (k c) -> p k c", c=BC)
    wr = w1t.rearrange("p (g k c) -> p g k c", k=K1, c=G1 * 128)

    with tile.TileContext(nc) as tc:
        with (
            tc.tile_pool(name="consts", bufs=1) as consts,
            tc.tile_pool(name="a1p", bufs=1) as a1p,
            tc.tile_pool(name="a2p", bufs=1) as a2p,
            tc.tile_pool(name="psum", bufs=5, space="PSUM") as psum,
            tc.tile_pool(name="psum_lg", bufs=2, space="PSUM") as psum_lg,
            tc.tile_pool(name="psum_tp", bufs=1, space="PSUM") as psum_tp,
            tc.tile_pool(name="w2s", bufs=2) as w2s,
        ):
            a1 = a1p.tile([128, NT2, 2, BC], FP8)
            a2 = a2p.tile([128, NT3, 2, BC], FP8)

            # fc4 logits accumulators [cls, batch-half], pre-zeroed, start=False
            lg = [psum_lg.tile([NCLS, 512], F32, tag="lg", name=f"lg{i}")
                  for i in range(NB)]

            # PE prewarm: ~16 dummy MMs fill the DMA-wait window at t~6-10us
            # so the HAM clock gate reaches 8/8 before real matmuls start.
            pw_w = consts.tile([128, NCLS], F16)
            pw_x = consts.tile([128, 512], F16)
            nc.vector.memset(pw_w, 0.0)
            nc.vector.memset(pw_x, 0.0)
            for _ in range(16):
                nc.tensor.matmul(lg[0], lhsT=pw_w, rhs=pw_x,
                                 start=True, stop=True, skip_group_check=True)

            # ---- fc1: fp16x2 exact split + sign threshold ----
            with tc.tile_pool(name="fc1res", bufs=1) as fc1res:
                xh = fc1res.tile([128, K1, BC], F16)
                w1s = fc1res.tile([128, NG1, K1, G1 * 128], FP8E5)

                # startup-critical DMAs, split across the sync + scalar
                # hardware queues; x n=0 chunks pace the first j-group.
                nc.sync.dma_start(out=xh[:, 0:2, 0:512], in_=xr[:, 0:2, 0:512])
                nc.sync.dma_start(out=xh[:, 2:5, 0:512], in_=xr[:, 2:5, 0:512])
                nc.sync.dma_start(out=xh[:, 5:9, 0:512], in_=xr[:, 5:9, 0:512])
                nc.sync.dma_start(out=xh[:, 9:K1, 0:512], in_=xr[:, 9:K1, 0:512])
                for g in range(NG1):
                    nc.scalar.dma_start(out=w1s[:, g], in_=wr[:, g])
                cv = consts.tile([128, NJ1 + 3 * NJ3], F32)
                nc.sync.dma_start(out=cv, in_=cvec[:, :])
                nt1 = cv[:, 0:NJ1]
                nt2 = cv[:, NJ1:NJ1 + NJ3]
                s3s = cv[:, NJ1 + NJ3:NJ1 + 2 * NJ3]
                c3s = cv[:, NJ1 + 2 * NJ3:NJ1 + 3 * NJ3]
                w4s = consts.tile([128, NJ3, NCLS], F16)
                nc.sync.dma_start(
                    out=w4s, in_=w4t.rearrange("p (j c) -> p j c", c=NCLS))
                b4s = consts.tile([NCLS, 1], F32)
                nc.sync.dma_start(out=b4s, in_=b4c[:, :])
                # x n=1 half: needed only after phase 0 (~65us in)
                nc.sync.dma_start(out=xh[:, 0:7, 512:1024],
                                  in_=xr[:, 0:7, 512:1024])
                nc.sync.dma_start(out=xh[:, 7:K1, 512:1024],
                                  in_=xr[:, 7:K1, 512:1024])
                for n in range(NB):
                    nc.vector.memset(lg[n], 0.0)

                # prefetch the first two fc2 weight slabs during fc1
                def w2_slab(jb):
                    wt = w2s.tile([128, NT2, 2, JB * 128], FP8, tag="w2t")
                    w2r = w2p[jb].rearrange("p (t i c) -> p t i c",
                                            i=2, c=JB * 128)
                    for tg in range(NT2 // 3):
                        nc.sync.dma_start(out=wt[:, 3 * tg:3 * tg + 3],
                                          in_=w2r[:, 3 * tg:3 * tg + 3])
                    return wt

                w2_pre = [w2_slab(0), w2_slab(1)]

                for n in range(NB):
                    for g in range(NG1):
                        pss = [psum.tile([128, 512], F32, tag="ps",
                                         name=f"f1_{n}_{g}_{i}")
                               for i in range(G1)]
                        for k in range(K1):
                            for j3 in range(G1):
                                nc.tensor.matmul(
                                    pss[j3],
                                    lhsT=w1s[:, g, k, j3 * 128:(j3 + 1) * 128],
                                    rhs=xh[:, k, n * 512:(n + 1) * 512],
                                    start=(k == 0),
                                    stop=(k == K1 - 1),
                                )
                        for j3 in range(G1):
                            j = g * G1 + j3
                            nc.scalar.activation(
                                out=a1[:, j // 2, j % 2, n * 512:(n + 1) * 512],
                                in_=pss[j3],
                                func=ACTF.Sign,
                                bias=nt1[:, j:j + 1],
                                scale=1.0,
                            )

            # ---- fc2: fp8 DoubleRow + sign threshold ----
            with (
                tc.tile_pool(name="w3s", bufs=2) as w3s,
                tc.tile_pool(name="h3p", bufs=3) as h3p,
            ):
                # prefetch the first two fc3 weight slabs (scalar queue)
                def w3_slab(jb):
                    wt = w3s.tile([128, NT3, 2, JB * 128], FP8, tag="w3t")
                    w3r = w3p[jb].rearrange("p (t i c) -> p t i c",
                                            i=2, c=JB * 128)
                    for tg in range(NT3 // 4):
                        nc.scalar.dma_start(out=wt[:, 4 * tg:4 * tg + 4],
                                            in_=w3r[:, 4 * tg:4 * tg + 4])
                    return wt

                w3_pre = [w3_slab(0), w3_slab(1)]

                for jb in range(NJ2 // JB):
                    wt = w2_pre[jb] if jb < 2 else w2_slab(jb)
                    for j in range(JB):
                        jj = jb * JB + j
                        for n in range(NB):
                            ps = psum.tile([128, 512], F32, tag="ps")
                            for t in range(NT2):
                                nc.tensor.matmul(
                                    ps,
                                    lhsT=wt[:, t, :, j * 128:(j + 1) * 128],
                                    rhs=a1[:, t, :, n * 512:(n + 1) * 512],
                                    start=(t == 0),
                                    stop=(t == NT2 - 1),
                                    perf_mode=DR,
                                )
                            nc.scalar.activation(
                                out=a2[:, jj // 2, jj % 2, n * 512:(n + 1) * 512],
                                in_=ps,
                                func=ACTF.Sign,
                                bias=nt2[:, jj:jj + 1],
                                scale=1.0,
                            )

                # ---- fc3 (fp8 DR) + bn3/hardtanh + fused fc4 (fp16) ----
                # fc4 MMs are emitted one DR group late so the bn3+clip
                # chain has a full group (~5us) of lead time.
                pend = None
                for jb in range(NJ3 // JB):
                    wt = w3_pre[jb] if jb < 2 else w3_slab(jb)
                    for j in range(JB):
                        jj = jb * JB + j
                        h3 = h3p.tile([128, BC], F16, tag="h3")
                        for n in range(NB):
                            ps = psum.tile([128, 512], F32, tag="ps")
                            for t in range(NT3):
                                nc.tensor.matmul(
                                    ps,
                                    lhsT=wt[:, t, :, j * 128:(j + 1) * 128],
                                    rhs=a2[:, t, :, n * 512:(n + 1) * 512],
                                    start=(t == 0),
                                    stop=(t == NT3 - 1),
                                    perf_mode=DR,
                                )
                            if pend is not None:
                                ph3, pn, pstop, pjj = pend
                                nc.tensor.matmul(
                                    lg[pn],
                                    lhsT=w4s[:, pjj, :],
                                    rhs=ph3[:, pn * 512:(pn + 1) * 512],
                                    start=False,
                                    stop=pstop,
                                    skip_group_check=True,
                                )
                            tmp = h3p.tile([128, 512], F32, tag="bn3tmp")
                            nc.scalar.activation(
                                out=tmp,
                                in_=ps,
                                func=ACTF.Identity,
                                bias=c3s[:, jj:jj + 1],
                                scale=s3s[:, jj:jj + 1],
                            )
                            nc.vector.tensor_scalar(
                                out=h3[:, n * 512:(n + 1) * 512],
                                in0=tmp,
                                scalar1=-1.0,
                                scalar2=1.0,
                                op0=ALU.max,
                                op1=ALU.min,
                            )
                            pend = (h3, n, jj == NJ3 - 1, jj)
                        h3 = None
                ph3, pn, pstop, pjj = pend
                nc.tensor.matmul(
                    lg[pn],
                    lhsT=w4s[:, pjj, :],
                    rhs=ph3[:, pn * 512:(pn + 1) * 512],
                    start=False,
                    stop=pstop,
                    skip_group_check=True,
                )

            # ---- epilogue: +b4, transpose [cls,b]->[b,cls], log_softmax ----
            with tc.tile_pool(name="epi", bufs=1) as epi:
                ident = consts.tile([NCLS, NCLS], F32)
                make_identity(nc, ident)
                lsb = epi.tile([NCLS, BC], F32, tag="lsb")
                tp = psum_tp.tile([128, NBCH, NCLS], F32, tag="tp")
                for n in range(NB):
                    nc.scalar.activation(
                        out=lsb[:, n * 512:(n + 1) * 512],
                        in_=lg[n],
                        func=ACTF.Identity,
                        bias=b4s[:, 0:1],
                        scale=1.0,
                    )
                    for b in range(4):
                        c = n * 4 + b
                        nc.tensor.transpose(
                            tp[:, c, :], lsb[:, c * 128:(c + 1) * 128], ident)
                # log_softmax without max-shift: logits are O(5), exp safe
                ex = epi.tile([128, NBCH, 10], F32, tag="ex")
                nc.scalar.activation(out=ex, in_=tp[:, :, 0:10], func=ACTF.Exp)
                sm = epi.tile([128, NBCH], F32, tag="sm")
                nc.vector.tensor_reduce(
                    out=sm, in_=ex, axis=mybir.AxisListType.X, op=ALU.add)
                lnt = epi.tile([128, NBCH], F32, tag="lnt")
                nc.scalar.activation(out=lnt, in_=sm, func=ACTF.Ln)
                res = epi.tile([128, NBCH, 10], F32, tag="res")
                for b in range(NBCH):
                    nc.vector.tensor_scalar(
                        out=res[:, b, :], in0=tp[:, b, 0:10],
                        scalar1=lnt[:, b:b + 1],
                        scalar2=None, op0=ALU.subtract,
                    )
                nc.sync.dma_start(
                    out=out.rearrange("(c p) f -> p c f", p=128), in_=res)

    _legalize_multiwait(nc)
    return nc


def _prep_inputs(inputs):
    f64 = {k: np.asarray(v, np.float64) for k, v in inputs.items()
           if k != "x"}
    x = np.asarray(inputs["x"], np.float32)

    s1 = f64["g1"] / np.sqrt(f64["v1"] + EPS)
    t1 = f64["m1"] - f64["b1"] - f64["be1"] / s1
    s2 = f64["g2"] / np.sqrt(f64["v2"] + EPS)
    t2 = f64["m2"] - f64["b2"] - f64["be2"] / s2
    s3 = f64["g3"] / np.sqrt(f64["v3"] + EPS)
    c3 = (f64["b3"] - f64["m3"]) * s3 + f64["be3"]

    shared = {}
    # cvec [128, 24+48*3]: per-feature consts arranged [partition, tile]
    cvec = np.zeros((128, NJ1 + 3 * NJ3), np.float32)
    cvec[:, 0:NJ1] = (-t1).astype(np.float32).reshape(NJ1, 128).T
    cvec[:, NJ1:NJ1 + NJ3] = (-t2).astype(np.float32).reshape(NJ3, 128).T
    cvec[:, NJ1 + NJ3:NJ1 + 2 * NJ3] = s3.astype(np.float32).reshape(NJ3, 128).T
    cvec[:, NJ1 + 2 * NJ3:] = c3.astype(np.float32).reshape(NJ3, 128).T
    shared["cvec"] = np.ascontiguousarray(cvec)

    b4p = np.zeros((NCLS, 1), np.float32)
    b4p[:10, 0] = np.asarray(inputs["b4"], np.float32)
    shared["b4c"] = b4p

    # w1: sign, transposed to [in, out]; k-tiles 0-5 = pass1 rows (+-1),
    # 6-11 = pass2 rows (+-2^-11), 12 = packed remainders of both passes.
    # Then permuted to j-group-major so each group is one contiguous DMA.
    w1b = np.sign(np.asarray(inputs["w1"], np.float32)).astype(np.float32)
    w1T = w1b.T  # [784, D1]
    w1f = np.zeros((128, K1, D1), np.float32)
    for k in range(K1F):
        w1f[:, k, :] = w1T[k * 128:(k + 1) * 128]
        w1f[:, k + K1F, :] = w1T[k * 128:(k + 1) * 128] * S2W
    w1f[0:16, 12, :] = w1T[768:784]
    w1f[16:32, 12, :] = w1T[768:784] * S2W
    w1e5 = w1f.astype(NP_FP8E5)
    shared["w1t"] = np.ascontiguousarray(
        w1e5.reshape(128, K1, NG1, G1 * 128).transpose(0, 2, 1, 3)
        .reshape(128, NG1 * K1 * G1 * 128))

    # w2/w3: sign -> DoubleRow pair layout, slab-contiguous per partition:
    # [njb, 128, nt*2*(JB*128)] fp8
    def pack_dr(w, njb_out):
        wT = np.sign(np.asarray(w, np.float32)).T  # [in, out]
        nin, nout = wT.shape
        nt = nin // 256
        a = wT.reshape(nt, 2, 128, nout).transpose(0, 2, 1, 3)  # [nt,128,2,out]
        a = a.reshape(nt, 128, 2, njb_out, JB * 128).transpose(3, 1, 0, 2, 4)
        # a: [njb, 128, nt, 2, JB*128]
        return np.ascontiguousarray(
            a.reshape(njb_out, 128, nt * 2 * JB * 128).astype(NP_FP8))

    shared["w2p"] = pack_dr(inputs["w2"], NJ2 // JB)
    shared["w3p"] = pack_dr(inputs["w3"], NJ3 // JB)

    # w4: [10, D2] -> fp16 [128, NJ3*NCLS]: elem [k, j*16+c] = w4[c, j*128+k]
    w4 = np.asarray(inputs["w4"], np.float32)
    w4tp = np.zeros((D2, NCLS), np.float32)
    w4tp[:, :10] = w4.T
    shared["w4t"] = np.ascontiguousarray(
        w4tp.reshape(NJ3, 128, NCLS).transpose(1, 0, 2)
        .reshape(128, NJ3 * NCLS).astype(np.float16))

    # x: transpose, fp16x2 split (pass2 scaled by 2^11); per-core layout
    # [128, K1*BC] with k-tile-major columns.
    xT = np.ascontiguousarray(x.T)  # [784, B]
    x1 = xT.astype(np.float16)
    x2s = ((xT - x1.astype(np.float32)) * S2L).astype(np.float16)
    per_core = []
    for cix in range(NCORES):
        sl = slice(cix * BC, (cix + 1) * BC)
        xa = np.zeros((K1, 128, BC), np.float16)
        xa[0:K1F] = x1[0:768, sl].reshape(K1F, 128, BC)
        xa[K1F:2 * K1F] = x2s[0:768, sl].reshape(K1F, 128, BC)
        xa[12, 0:16] = x1[768:784, sl]
        xa[12, 16:32] = x2s[768:784, sl]
        m = dict(shared)
        m["xht"] = np.ascontiguousarray(
            xa.transpose(1, 0, 2).reshape(128, K1 * BC))
        per_core.append(m)
    return per_core


_NC_CACHE = None


def kernel(**inputs):
    global _NC_CACHE, LAST_EXEC_NS
    if _NC_CACHE is None:
        _NC_CACHE = _build_nc()
    nc = _NC_CACHE
    in_maps = _prep_inputs(inputs)
    kwargs = {}
    if TRACE:
        _install_ntff_shim()
        kwargs = dict(trace=True, tmpdir=TRACE_DIR)
    res = None
    outs = None
    for attempt in range(3):
        try:
            res = run_bass_kernel_spmd(nc, in_maps, core_ids=list(range(NCORES)),
                                       **kwargs)
            outs = [np.asarray(res.results[c]["out"]) for c in range(NCORES)]
            break
        except Exception:
            if attempt == 2:
                raise
    LAST_EXEC_NS = res.exec_time_ns
    return np.concatenate(outs, axis=0)


def _install_ntff_shim():
    """antenv.axon_hooks shim so trace=True works under axon (profiling only)."""
    import contextlib
    import ctypes
    import types

    if "antenv.axon_hooks" in sys.modules:
        return
    try:
        lib = ctypes.CDLL("/opt/axon/libaxon_pjrt.so")
        lib.axon_start_nrt_profile.argtypes = [
            ctypes.POINTER(ctypes.c_int64), ctypes.c_size_t]
        lib.axon_start_nrt_profile.restype = ctypes.c_int64
        lib.axon_stop_nrt_profile.argtypes = [ctypes.c_char_p]
        lib.axon_stop_nrt_profile.restype = ctypes.c_int64
    except (OSError, AttributeError):
        return

    @contextlib.contextmanager
    def _hook(output_dir, device_ids):
        import jax
        jax.devices()
        if device_ids:
            ids = (ctypes.c_int64 * len(device_ids))(*device_ids)
            rc = lib.axon_start_nrt_profile(ids, len(device_ids))
        else:
            rc = lib.axon_start_nrt_profile(None, 0)
        if rc != 0:
            raise RuntimeError(f"axon_start_nrt_profile rc={rc}")
        try:
            yield
        finally:
            n = lib.axon_stop_nrt_profile(str(output_dir).encode())
            print(f"ntff: {n} profile file(s) -> {output_dir}", file=sys.stderr)

    mod = types.ModuleType("antenv.axon_hooks")
    mod.get_axon_ntff_profile_hook = lambda: _hook
    mod.set_axon_ntff_profile_hook = lambda h: None
    sys.modules["antenv.axon_hooks"] = mod


# revision 12
# speedup vs baseline: 1.2496x; 1.0187x over previous
"""Trainium2 Bass kernel for the Binary-MLP (nn_Binary0) problem.

Strategy (8-way batch-parallel, 1024 rows/core):
  fc1: h1 = x @ sign(w1).T        -- fp16x2 split of x (exact to ~2^-22):
       pass1 rhs = fp16(x), lhsT = +-1; pass2 rhs = fp16((x-x1)*2^11),
       lhsT = +-2^-11 (both weight scales exact in fp8e5m2). 13 k-tiles
       vs 19 for the old bf16x3 split. k-outer loop over j-groups of 3
       so the PE saturates as soon as the first x k-tiles land.
       a1 = sign(h1 - t1)          -- thresholds fold bias+BN (host fp64)
  fc2: h2 = a1 @ sign(w2).T        -- fp8 DoubleRow (exact: +-1 products)
       a2 = sign(h2 - t2)
  fc3: h3 = a2 @ sign(w3).T        -- fp8 DoubleRow
       h3c = clip(h3*s3 + c3, -1, 1) -> fp16
  fc4: logits.T = w4 @ h3c         -- fp16 (216ns/MM vs 380 for f32r),
                                      fused into fc3 loop, [cls, batch]
  out = log_softmax(logits)        -- PE-transpose, free-dim reduce,
                                      single batched output DMA

DMA: triggers cost ~650ns serially per queue (~200GB/s per queue for
128KB transfers) -> few big chunked DMAs, split across the sync and
scalar (Activation) hardware queues at startup.
"""
import sys

for _p in ("/opt/trn_rl_repo",):
    if _p not in sys.path:
        sys.path.insert(0, _p)

import numpy as np
import ml_dtypes

import concourse.bass as bass
import concourse.tile as tile
import concourse.mybir as mybir
from concourse.bass_utils import run_bass_kernel_spmd
from concourse.masks import make_identity

F32 = mybir.dt.float32
F16 = mybir.dt.float16
BF16 = mybir.dt.bfloat16
FP8 = mybir.dt.float8e4
FP8E5 = mybir.dt.float8e5
NP_FP8 = mybir.dt.np(FP8)
NP_FP8E5 = mybir.dt.np(FP8E5)

EPS = 1e-5
NCORES = 8
B = 8192
BC = B // NCORES            # 1024 batch rows per core
D0, D1, D2 = 784, 3072, 6144
K1 = 13                     # fc1 k-tiles: 6 pass1 + 6 pass2 + 1 packed
K1F = 6                     # full 128-row k-tiles per pass (768 rows)
NJ1 = D1 // 128             # 24 fc1 output feature tiles
G1 = 3                      # fc1 j-tiles per psum group
NG1 = NJ1 // G1             # 8 groups
NT2 = D1 // 256             # 12 fc2 DoubleRow contraction tiles
NJ2 = D2 // 128             # 48
NT3 = D2 // 256             # 24 fc3 DoubleRow contraction tiles
NJ3 = D2 // 128             # 48
JB = 4                      # j-tiles per streamed weight slab
NB = 2                      # 512-wide batch halves of BC
NBCH = BC // 128            # 8 batch chunks
NCLS = 16                   # padded class dim (10 real)
S2L = 2.0 ** 11             # pass2 rhs scale
S2W = 2.0 ** -11            # pass2 weight scale

TRACE = False               # test.py sets True for profiling
TRACE_DIR = None
LAST_EXEC_NS = None

DR = mybir.MatmulPerfMode.DoubleRow
ACTF = mybir.ActivationFunctionType
ALU = mybir.AluOpType


def _legalize_multiwait(nc):
    """This container's walrus build rejects >1 sync-wait on one instruction
    (codegen 'Too many sync wait commands'); split extra waits into NoOps."""
    n = 0
    for f in nc.m.functions:
        for blk in f.blocks:
            insts = list(blk.instructions)
            new = []
            changed = False
            for ins in insts:
                si = ins.sync_info
                waits = list(si.on_wait) if (si is not None and si.on_wait) else []
                if len(waits) > 1:
                    for k, w in enumerate(waits[:-1]):
                        nop = mybir.InstNoOp(name=f"{ins.name}-sw{k}", ins=[], outs=[])
                        nop.engine = ins.engine
                        nop.sync_info = mybir.SyncInfo(on_wait=[w], on_update=[])
                        new.append(nop)
                        n += 1
                    ins.sync_info = mybir.SyncInfo(
                        on_wait=[waits[-1]], on_update=list(si.on_update or [])
                    )
                    changed = True
                new.append(ins)
            if changed:
                blk.instructions = new
    return n


def _build_nc():
    nc = bass.Bass("TRN2")

    xht = nc.dram_tensor("xht", [128, K1 * BC], F16, kind="ExternalInput")
    w1t = nc.dram_tensor("w1t", [128, NG1 * K1 * G1 * 128], FP8E5,
                         kind="ExternalInput")
    w2p = nc.dram_tensor("w2p", [NJ2 // JB, 128, NT2 * 2 * JB * 128], FP8,
                         kind="ExternalInput")
    w3p = nc.dram_tensor("w3p", [NJ3 // JB, 128, NT3 * 2 * JB * 128], FP8,
                         kind="ExternalInput")
    w4t = nc.dram_tensor("w4t", [128, NJ3 * NCLS], F16, kind="ExternalInput")
    # cvec columns: [0:24]=-t1, [24:72]=-t2, [72:120]=s3, [120:168]=c3
    cvec = nc.dram_tensor("cvec", [128, NJ1 + 3 * NJ3], F32, kind="ExternalInput")
    b4c = nc.dram_tensor("b4c", [NCLS, 1], F32, kind="ExternalInput")
    out = nc.dram_tensor("out", [BC, 10], F32, kind="ExternalOutput")

    xr = xht.rearrange("p (k c) -> p k c", c=BC)
    wr = w1t.rearrange("p (g k c) -> p g k c", k=K1, c=G1 * 128)

    with tile.TileContext(nc) as tc:
        with (
            tc.tile_pool(name="consts", bufs=1) as consts,
            tc.tile_pool(name="a1p", bufs=1) as a1p,
            tc.tile_pool(name="a2p", bufs=1) as a2p,
            tc.tile_pool(name="psum", bufs=5, space="PSUM") as psum,
            tc.tile_pool(name="psum_lg", bufs=2, space="PSUM") as psum_lg,
            tc.tile_pool(name="psum_tp", bufs=1, space="PSUM") as psum_tp,
            tc.tile_pool(name="w2s", bufs=2) as w2s,
        ):
            a1 = a1p.tile([128, NT2, 2, BC], FP8)
            a2 = a2p.tile([128, NT3, 2, BC], FP8)

            # fc4 logits accumulators [cls, batch-half], pre-zeroed, start=False
            lg = [psum_lg.tile([NCLS, 512], F32, tag="lg", name=f"lg{i}")
                  for i in range(NB)]

            # PE prewarm: ~16 dummy MMs fill the DMA-wait window at t~6-10us
            # so the HAM clock gate reaches 8/8 before real matmuls start.
            pw_w = consts.tile([128, NCLS], F16)
            pw_x = consts.tile([128, 512], F16)
            nc.vector.memset(pw_w, 0.0)
            nc.vector.memset(pw_x, 0.0)
            for _ in range(16):
                nc.tensor.matmul(lg[0], lhsT=pw_w, rhs=pw_x,
                                 start=True, stop=True, skip_group_check=True)

            # ---- fc1: fp16x2 exact split + sign threshold ----
            with tc.tile_pool(name="fc1res", bufs=1) as fc1res:
                xh = fc1res.tile([128, K1, BC], F16)
                w1s = fc1res.tile([128, NG1, K1, G1 * 128], FP8E5)

                # startup-critical DMAs, split across the sync + scalar
                # hardware queues; x n=0 chunks pace the first j-group.
                nc.sync.dma_start(out=xh[:, 0:2, 0:512], in_=xr[:, 0:2, 0:512])
                nc.sync.dma_start(out=xh[:, 2:5, 0:512], in_=xr[:, 2:5, 0:512])
                nc.sync.dma_start(out=xh[:, 5:9, 0:512], in_=xr[:, 5:9, 0:512])
                nc.sync.dma_start(out=xh[:, 9:K1, 0:512], in_=xr[:, 9:K1, 0:512])
                for g in range(NG1):
                    nc.scalar.dma_start(out=w1s[:, g], in_=wr[:, g])
                cv = consts.tile([128, NJ1 + 3 * NJ3], F32)
                nc.sync.dma_start(out=cv, in_=cvec[:, :])
                nt1 = cv[:, 0:NJ1]
                nt2 = cv[:, NJ1:NJ1 + NJ3]
                s3s = cv[:, NJ1 + NJ3:NJ1 + 2 * NJ3]
                c3s = cv[:, NJ1 + 2 * NJ3:NJ1 + 3 * NJ3]
                w4s = consts.tile([128, NJ3, NCLS], F16)
                nc.sync.dma_start(
                    out=w4s, in_=w4t.rearrange("p (j c) -> p j c", c=NCLS))
                b4s = consts.tile([NCLS, 1], F32)
                nc.sync.dma_start(out=b4s, in_=b4c[:, :])
                # x n=1 half: needed only after phase 0 (~65us in)
                nc.sync.dma_start(out=xh[:, 0:7, 512:1024],
                                  in_=xr[:, 0:7, 512:1024])
                nc.sync.dma_start(out=xh[:, 7:K1, 512:1024],
                                  in_=xr[:, 7:K1, 512:1024])
                for n in range(NB):
                    nc.vector.memset(lg[n], 0.0)

                # prefetch the first two fc2 weight slabs during fc1
                def w2_slab(jb):
                    wt = w2s.tile([128, NT2, 2, JB * 128], FP8, tag="w2t")
                    w2r = w2p[jb].rearrange("p (t i c) -> p t i c",
                                            i=2, c=JB * 128)
                    for tg in range(NT2 // 3):
                        nc.sync.dma_start(out=wt[:, 3 * tg:3 * tg + 3],
                                          in_=w2r[:, 3 * tg:3 * tg + 3])
                    return wt

                w2_pre = [w2_slab(0), w2_slab(1)]

                for n in range(NB):
                    for g in range(NG1):
                        pss = [psum.tile([128, 512], F32, tag="ps",
                                         name=f"f1_{n}_{g}_{i}")
                               for i in range(G1)]
                        for k in range(K1 - 1):
                            for j3 in range(G1):
                                nc.tensor.matmul(
                                    pss[j3],
                                    lhsT=w1s[:, g, k, j3 * 128:(j3 + 1) * 128],
                                    rhs=xh[:, k, n * 512:(n + 1) * 512],
                                    start=(k == 0),
                                    stop=False,
                                )
                        # packed 32-row remainder tile: the 3 j-tiles run
                        # CONCURRENTLY in distinct PE row-groups (data is
                        # host-replicated at partition offsets 0/32/64).
                        for j3 in range(G1):
                            p0 = 32 * j3
                            nc.tensor.matmul(
                                pss[j3],
                                lhsT=w1s[p0:p0 + 32, g, K1 - 1,
                                         j3 * 128:(j3 + 1) * 128],
                                rhs=xh[p0:p0 + 32, K1 - 1,
                                       n * 512:(n + 1) * 512],
                                start=False,
                                stop=True,
                                tile_position=(p0, 0),
                            )
                        for j3 in range(G1):
                            j = g * G1 + j3
                            nc.scalar.activation(
                                out=a1[:, j // 2, j % 2, n * 512:(n + 1) * 512],
                                in_=pss[j3],
                                func=ACTF.Sign,
                                bias=nt1[:, j:j + 1],
                                scale=1.0,
                            )

            # ---- fc2: fp8 DoubleRow + sign threshold ----
            with (
                tc.tile_pool(name="w3s", bufs=2) as w3s,
                tc.tile_pool(name="h3p", bufs=6) as h3p,
            ):
                # prefetch the first two fc3 weight slabs (scalar queue)
                def w3_slab(jb):
                    wt = w3s.tile([128, NT3, 2, JB * 128], FP8, tag="w3t")
                    w3r = w3p[jb].rearrange("p (t i c) -> p t i c",
                                            i=2, c=JB * 128)
                    for tg in range(NT3 // 4):
                        nc.scalar.dma_start(out=wt[:, 4 * tg:4 * tg + 4],
                                            in_=w3r[:, 4 * tg:4 * tg + 4])
                    return wt

                w3_pre = [w3_slab(0), w3_slab(1)]

                for jb in range(NJ2 // JB):
                    wt = w2_pre[jb] if jb < 2 else w2_slab(jb)
                    for j in range(JB):
                        jj = jb * JB + j
                        for n in range(NB):
                            ps = psum.tile([128, 512], F32, tag="ps")
                            for t in range(NT2):
                                nc.tensor.matmul(
                                    ps,
                                    lhsT=wt[:, t, :, j * 128:(j + 1) * 128],
                                    rhs=a1[:, t, :, n * 512:(n + 1) * 512],
                                    start=(t == 0),
                                    stop=(t == NT2 - 1),
                                    perf_mode=DR,
                                )
                            nc.scalar.activation(
                                out=a2[:, jj // 2, jj % 2, n * 512:(n + 1) * 512],
                                in_=ps,
                                func=ACTF.Sign,
                                bias=nt2[:, jj:jj + 1],
                                scale=1.0,
                            )

                # ---- fc3 (fp8 DR) + bn3/hardtanh + fused fc4 (fp16) ----
                # fc4 MMs are batched per weight slab (4 jj x 2 n emitted
                # back-to-back one slab late): consecutive M=16 matmuls
                # pipeline, amortizing the array switch cost, and the
                # bn3+clip chains get a full slab of lead time.
                pend4 = []

                def flush_fc4():
                    for n in range(NB):
                        for ph3, pjj in pend4:
                            nc.tensor.matmul(
                                lg[n],
                                lhsT=w4s[:, pjj, :],
                                rhs=ph3[:, n * 512:(n + 1) * 512],
                                start=False,
                                stop=(pjj == NJ3 - 1),
                                skip_group_check=True,
                            )
                    pend4.clear()

                for jb in range(NJ3 // JB):
                    wt = w3_pre[jb] if jb < 2 else w3_slab(jb)
                    for j in range(JB):
                        jj = jb * JB + j
                        h3 = h3p.tile([128, BC], F16, tag="h3")
                        for n in range(NB):
                            ps = psum.tile([128, 512], F32, tag="ps")
                            for t in range(NT3):
                                nc.tensor.matmul(
                                    ps,
                                    lhsT=wt[:, t, :, j * 128:(j + 1) * 128],
                                    rhs=a2[:, t, :, n * 512:(n + 1) * 512],
                                    start=(t == 0),
                                    stop=(t == NT3 - 1),
                                    perf_mode=DR,
                                )
                            if j == 0 and n == 1 and pend4:
                                flush_fc4()
                            tmp = h3p.tile([128, 512], F32, tag="bn3tmp")
                            nc.scalar.activation(
                                out=tmp,
                                in_=ps,
                                func=ACTF.Identity,
                                bias=c3s[:, jj:jj + 1],
                                scale=s3s[:, jj:jj + 1],
                            )
                            nc.vector.tensor_scalar(
                                out=h3[:, n * 512:(n + 1) * 512],
                                in0=tmp,
                                scalar1=-1.0,
                                scalar2=1.0,
                                op0=ALU.max,
                                op1=ALU.min,
                            )
                        pend4.append((h3, jj))
                flush_fc4()

            # ---- epilogue: +b4, transpose [cls,b]->[b,cls], log_softmax ----
            with tc.tile_pool(name="epi", bufs=1) as epi:
                ident = consts.tile([NCLS, NCLS], F32)
                make_identity(nc, ident)
                lsb = epi.tile([NCLS, BC], F32, tag="lsb")
                tp = psum_tp.tile([128, NBCH, NCLS], F32, tag="tp")
                for n in range(NB):
                    nc.scalar.activation(
                        out=lsb[:, n * 512:(n + 1) * 512],
                        in_=lg[n],
                        func=ACTF.Identity,
                        bias=b4s[:, 0:1],
                        scale=1.0,
                    )
                    for b in range(4):
                        c = n * 4 + b
                        nc.tensor.transpose(
                            tp[:, c, :], lsb[:, c * 128:(c + 1) * 128], ident)
                # log_softmax without max-shift: logits are O(5), exp safe
                ex = epi.tile([128, NBCH, 10], F32, tag="ex")
                nc.scalar.activation(out=ex, in_=tp[:, :, 0:10], func=ACTF.Exp)
                sm = epi.tile([128, NBCH], F32, tag="sm")
                nc.vector.tensor_reduce(
                    out=sm, in_=ex, axis=mybir.AxisListType.X, op=ALU.add)
                lnt = epi.tile([128, NBCH], F32, tag="lnt")
                nc.scalar.activation(out=lnt, in_=sm, func=ACTF.Ln)
                res = epi.tile([128, NBCH, 10], F32, tag="res")
                for b in range(NBCH):
                    nc.vector.tensor_scalar(
                        out=res[:, b, :], in0=tp[:, b, 0:10],
                        scalar1=lnt[:, b:b + 1],
                        scalar2=None, op0=ALU.subtract,
                    )
                nc.sync.dma_start(
                    out=out.rearrange("(c p) f -> p c f", p=128), in_=res)

    _legalize_multiwait(nc)
    return nc


def _prep_inputs(inputs):
    f64 = {k: np.asarray(v, np.float64) for k, v in inputs.items()
           if k != "x"}
    x = np.asarray(inputs["x"], np.float32)

    s1 = f64["g1"] / np.sqrt(f64["v1"] + EPS)
    t1 = f64["m1"] - f64["b1"] - f64["be1"] / s1
    s2 = f64["g2"] / np.sqrt(f64["v2"] + EPS)
    t2 = f64["m2"] - f64["b2"] - f64["be2"] / s2
    s3 = f64["g3"] / np.sqrt(f64["v3"] + EPS)
    c3 = (f64["b3"] - f64["m3"]) * s3 + f64["be3"]

    shared = {}
    # cvec [128, 24+48*3]: per-feature consts arranged [partition, tile]
    cvec = np.zeros((128, NJ1 + 3 * NJ3), np.float32)
    cvec[:, 0:NJ1] = (-t1).astype(np.float32).reshape(NJ1, 128).T
    cvec[:, NJ1:NJ1 + NJ3] = (-t2).astype(np.float32).reshape(NJ3, 128).T
    cvec[:, NJ1 + NJ3:NJ1 + 2 * NJ3] = s3.astype(np.float32).reshape(NJ3, 128).T
    cvec[:, NJ1 + 2 * NJ3:] = c3.astype(np.float32).reshape(NJ3, 128).T
    shared["cvec"] = np.ascontiguousarray(cvec)

    b4p = np.zeros((NCLS, 1), np.float32)
    b4p[:10, 0] = np.asarray(inputs["b4"], np.float32)
    shared["b4c"] = b4p

    # w1: sign, transposed to [in, out]; k-tiles 0-5 = pass1 rows (+-1),
    # 6-11 = pass2 rows (+-2^-11), 12 = packed remainders of both passes.
    # Then permuted to j-group-major so each group is one contiguous DMA.
    w1b = np.sign(np.asarray(inputs["w1"], np.float32)).astype(np.float32)
    w1T = w1b.T  # [784, D1]
    w1f = np.zeros((128, K1, D1), np.float32)
    for k in range(K1F):
        w1f[:, k, :] = w1T[k * 128:(k + 1) * 128]
        w1f[:, k + K1F, :] = w1T[k * 128:(k + 1) * 128] * S2W
    w1f[0:16, 12, :] = w1T[768:784]
    w1f[16:32, 12, :] = w1T[768:784] * S2W
    for r in range(1, 3):  # replicate packed tile for PE row-group tiling
        w1f[32 * r:32 * r + 32, 12, :] = w1f[0:32, 12, :]
    w1e5 = w1f.astype(NP_FP8E5)
    shared["w1t"] = np.ascontiguousarray(
        w1e5.reshape(128, K1, NG1, G1 * 128).transpose(0, 2, 1, 3)
        .reshape(128, NG1 * K1 * G1 * 128))

    # w2/w3: sign -> DoubleRow pair layout, slab-contiguous per partition:
    # [njb, 128, nt*2*(JB*128)] fp8
    def pack_dr(w, njb_out):
        wT = np.sign(np.asarray(w, np.float32)).T  # [in, out]
        nin, nout = wT.shape
        nt = nin // 256
        a = wT.reshape(nt, 2, 128, nout).transpose(0, 2, 1, 3)  # [nt,128,2,out]
        a = a.reshape(nt, 128, 2, njb_out, JB * 128).transpose(3, 1, 0, 2, 4)
        # a: [njb, 128, nt, 2, JB*128]
        return np.ascontiguousarray(
            a.reshape(njb_out, 128, nt * 2 * JB * 128).astype(NP_FP8))

    shared["w2p"] = pack_dr(inputs["w2"], NJ2 // JB)
    shared["w3p"] = pack_dr(inputs["w3"], NJ3 // JB)

    # w4: [10, D2] -> fp16 [128, NJ3*NCLS]: elem [k, j*16+c] = w4[c, j*128+k]
    w4 = np.asarray(inputs["w4"], np.float32)
    w4tp = np.zeros((D2, NCLS), np.float32)
    w4tp[:, :10] = w4.T
    shared["w4t"] = np.ascontiguousarray(
        w4tp.reshape(NJ3, 128, NCLS).transpose(1, 0, 2)
        .reshape(128, NJ3 * NCLS).astype(np.float16))

    # x: transpose, fp16x2 split (pass2 scaled by 2^11); per-core layout
    # [128, K1*BC] with k-tile-major columns.
    xT = np.ascontiguousarray(x.T)  # [784, B]
    x1 = xT.astype(np.float16)
    x2s = ((xT - x1.astype(np.float32)) * S2L).astype(np.float16)
    per_core = []
    for cix in range(NCORES):
        sl = slice(cix * BC, (cix + 1) * BC)
        xa = np.zeros((K1, 128, BC), np.float16)
        xa[0:K1F] = x1[0:768, sl].reshape(K1F, 128, BC)
        xa[K1F:2 * K1F] = x2s[0:768, sl].reshape(K1F, 128, BC)
        xa[12, 0:16] = x1[768:784, sl]
        xa[12, 16:32] = x2s[768:784, sl]
        for r in range(1, 3):
            xa[12, 32 * r:32 * r + 32] = xa[12, 0:32]
        m = dict(shared)
        m["xht"] = np.ascontiguousarray(
            xa.transpose(1, 0, 2).reshape(128, K1 * BC))
        per_core.append(m)
    return per_core


_NC_CACHE = None


def kernel(**inputs):
    global _NC_CACHE, LAST_EXEC_NS
    if _NC_CACHE is None:
        _NC_CACHE = _build_nc()
    nc = _NC_CACHE
    in_maps = _prep_inputs(inputs)
    kwargs = {}
    if TRACE:
        _install_ntff_shim()
        kwargs = dict(trace=True, tmpdir=TRACE_DIR)
    res = None
    outs = None
    for attempt in range(3):
        try:
            res = run_bass_kernel_spmd(nc, in_maps, core_ids=list(range(NCORES)),
                                       **kwargs)
            outs = [np.asarray(res.results[c]["out"]) for c in range(NCORES)]
            break
        except Exception:
            if attempt == 2:
                raise
    LAST_EXEC_NS = res.exec_time_ns
    return np.concatenate(outs, axis=0)


def _install_ntff_shim():
    """antenv.axon_hooks shim so trace=True works under axon (profiling only)."""
    import contextlib
    import ctypes
    import types

    if "antenv.axon_hooks" in sys.modules:
        return
    try:
        lib = ctypes.CDLL("/opt/axon/libaxon_pjrt.so")
        lib.axon_start_nrt_profile.argtypes = [
            ctypes.POINTER(ctypes.c_int64), ctypes.c_size_t]
        lib.axon_start_nrt_profile.restype = ctypes.c_int64
        lib.axon_stop_nrt_profile.argtypes = [ctypes.c_char_p]
        lib.axon_stop_nrt_profile.restype = ctypes.c_int64
    except (OSError, AttributeError):
        return

    @contextlib.contextmanager
    def _hook(output_dir, device_ids):
        import jax
        jax.devices()
        if device_ids:
            ids = (ctypes.c_int64 * len(device_ids))(*device_ids)
            rc = lib.axon_start_nrt_profile(ids, len(device_ids))
        else:
            rc = lib.axon_start_nrt_profile(None, 0)
        if rc != 0:
            raise RuntimeError(f"axon_start_nrt_profile rc={rc}")
        try:
            yield
        finally:
            n = lib.axon_stop_nrt_profile(str(output_dir).encode())
            print(f"ntff: {n} profile file(s) -> {output_dir}", file=sys.stderr)

    mod = types.ModuleType("antenv.axon_hooks")
    mod.get_axon_ntff_profile_hook = lambda: _hook
    mod.set_axon_ntff_profile_hook = lambda h: None
    sys.modules["antenv.axon_hooks"] = mod


# revision 16
# speedup vs baseline: 1.2539x; 1.0035x over previous
"""Trainium2 Bass kernel for the Binary-MLP (nn_Binary0) problem.

Strategy (8-way batch-parallel, 1024 rows/core):
  fc1: h1 = x @ sign(w1).T        -- fp16x2 split of x (exact to ~2^-22):
       pass1 rhs = fp16(x), lhsT = +-1; pass2 rhs = fp16((x-x1)*2^11),
       lhsT = +-2^-11 (both weight scales exact in fp8e5m2). 13 k-tiles
       vs 19 for the old bf16x3 split. k-outer loop over j-groups of 3
       so the PE saturates as soon as the first x k-tiles land.
       a1 = sign(h1 - t1)          -- thresholds fold bias+BN (host fp64)
  fc2: h2 = a1 @ sign(w2).T        -- fp8 DoubleRow (exact: +-1 products)
       a2 = sign(h2 - t2)
  fc3: h3 = a2 @ sign(w3).T        -- fp8 DoubleRow
       h3c = clip(h3*s3 + c3, -1, 1) -> fp16
  fc4: logits.T = w4 @ h3c         -- fp16 (216ns/MM vs 380 for f32r),
                                      fused into fc3 loop, [cls, batch]
  out = log_softmax(logits)        -- PE-transpose, free-dim reduce,
                                      single batched output DMA

DMA: triggers cost ~650ns serially per queue (~200GB/s per queue for
128KB transfers) -> few big chunked DMAs, split across the sync and
scalar (Activation) hardware queues at startup.
"""
import sys

for _p in ("/opt/trn_rl_repo",):
    if _p not in sys.path:
        sys.path.insert(0, _p)

import numpy as np
import ml_dtypes

import concourse.bass as bass
import concourse.tile as tile
import concourse.mybir as mybir
from concourse.bass_utils import run_bass_kernel_spmd
from concourse.masks import make_identity

F32 = mybir.dt.float32
F16 = mybir.dt.float16
BF16 = mybir.dt.bfloat16
FP8 = mybir.dt.float8e4
FP8E5 = mybir.dt.float8e5
NP_FP8 = mybir.dt.np(FP8)
NP_FP8E5 = mybir.dt.np(FP8E5)

EPS = 1e-5
NCORES = 8
B = 8192
BC = B // NCORES            # 1024 batch rows per core
D0, D1, D2 = 784, 3072, 6144
K1 = 13                     # fc1 k-tiles: 6 pass1 + 6 pass2 + 1 packed
K1F = 6                     # full 128-row k-tiles per pass (768 rows)
NJ1 = D1 // 128             # 24 fc1 output feature tiles
G1 = 2                      # fc1 j-tiles per psum group (2 -> psum bank
                            # reuse distance spans 2 full groups: no stalls)
NG1 = NJ1 // G1             # 12 groups
NT2 = D1 // 256             # 12 fc2 DoubleRow contraction tiles
NJ2 = D2 // 128             # 48
NT3 = D2 // 256             # 24 fc3 DoubleRow contraction tiles
NJ3 = D2 // 128             # 48
JB = 4                      # j-tiles per streamed weight slab
NB = 2                      # 512-wide batch halves of BC
NBCH = BC // 128            # 8 batch chunks
NCLS = 16                   # padded class dim (10 real)
S2L = 2.0 ** 11             # pass2 rhs scale
S2W = 2.0 ** -11            # pass2 weight scale

TRACE = False               # test.py sets True for profiling
TRACE_DIR = None
LAST_EXEC_NS = None

DR = mybir.MatmulPerfMode.DoubleRow
ACTF = mybir.ActivationFunctionType
ALU = mybir.AluOpType


def _legalize_multiwait(nc):
    """This container's walrus build rejects >1 sync-wait on one instruction
    (codegen 'Too many sync wait commands'); split extra waits into NoOps."""
    n = 0
    for f in nc.m.functions:
        for blk in f.blocks:
            insts = list(blk.instructions)
            new = []
            changed = False
            for ins in insts:
                si = ins.sync_info
                waits = list(si.on_wait) if (si is not None and si.on_wait) else []
                if len(waits) > 1:
                    for k, w in enumerate(waits[:-1]):
                        nop = mybir.InstNoOp(name=f"{ins.name}-sw{k}", ins=[], outs=[])
                        nop.engine = ins.engine
                        nop.sync_info = mybir.SyncInfo(on_wait=[w], on_update=[])
                        new.append(nop)
                        n += 1
                    ins.sync_info = mybir.SyncInfo(
                        on_wait=[waits[-1]], on_update=list(si.on_update or [])
                    )
                    changed = True
                new.append(ins)
            if changed:
                blk.instructions = new
    return n


def _build_nc():
    nc = bass.Bass("TRN2")

    xht = nc.dram_tensor("xht", [128, K1 * BC], F16, kind="ExternalInput")
    w1t = nc.dram_tensor("w1t", [128, NG1 * K1 * G1 * 128], FP8E5,
                         kind="ExternalInput")
    w2p = nc.dram_tensor("w2p", [NJ2 // JB, 128, NT2 * 2 * JB * 128], FP8,
                         kind="ExternalInput")
    w3p = nc.dram_tensor("w3p", [NJ3 // JB, 128, NT3 * 2 * JB * 128], FP8,
                         kind="ExternalInput")
    w4t = nc.dram_tensor("w4t", [128, NJ3 * NCLS], F16, kind="ExternalInput")
    # cvec columns: [0:24]=-t1, [24:72]=-t2, [72:120]=s3, [120:168]=c3
    cvec = nc.dram_tensor("cvec", [128, NJ1 + 3 * NJ3], F32, kind="ExternalInput")
    b4c = nc.dram_tensor("b4c", [NCLS, 1], F32, kind="ExternalInput")
    out = nc.dram_tensor("out", [BC, 10], F32, kind="ExternalOutput")

    xr = xht.rearrange("p (k c) -> p k c", c=BC)
    wr = w1t.rearrange("p (g k c) -> p g k c", k=K1, c=G1 * 128)

    with tile.TileContext(nc) as tc:
        with (
            tc.tile_pool(name="consts", bufs=1) as consts,
            tc.tile_pool(name="a1p", bufs=1) as a1p,
            tc.tile_pool(name="a2p", bufs=1) as a2p,
            tc.tile_pool(name="psum", bufs=5, space="PSUM") as psum,
            tc.tile_pool(name="psum_lg", bufs=2, space="PSUM") as psum_lg,
            tc.tile_pool(name="psum_tp", bufs=1, space="PSUM") as psum_tp,
            tc.tile_pool(name="w2s", bufs=2) as w2s,
        ):
            a1 = a1p.tile([128, NT2, 2, BC], FP8)
            a2 = a2p.tile([128, NT3, 2, BC], FP8)

            # fc4 logits accumulators [cls, batch-half], pre-zeroed, start=False
            lg = [psum_lg.tile([NCLS, 512], F32, tag="lg", name=f"lg{i}")
                  for i in range(NB)]

            # PE prewarm: ~16 dummy MMs fill the DMA-wait window at t~6-10us
            # so the HAM clock gate reaches 8/8 before real matmuls start.
            pw_w = consts.tile([128, NCLS], F16)
            pw_x = consts.tile([128, 512], F16)
            nc.vector.memset(pw_w, 0.0)
            nc.vector.memset(pw_x, 0.0)
            for _ in range(28):
                nc.tensor.matmul(lg[0], lhsT=pw_w, rhs=pw_x,
                                 start=True, stop=True, skip_group_check=True)

            # ---- fc1: fp16x2 exact split + sign threshold ----
            with tc.tile_pool(name="fc1res", bufs=1) as fc1res:
                xh = fc1res.tile([128, K1, BC], F16)
                w1s = fc1res.tile([128, NG1, K1, G1 * 128], FP8E5)

                # startup-critical DMAs, split across the sync + scalar
                # hardware queues; x n=0 chunks pace the first j-group.
                nc.sync.dma_start(out=xh[:, 0:2, 0:512], in_=xr[:, 0:2, 0:512])
                nc.sync.dma_start(out=xh[:, 2:4, 0:512], in_=xr[:, 2:4, 0:512])
                nc.sync.dma_start(out=xh[:, 4:6, 0:512], in_=xr[:, 4:6, 0:512])
                nc.sync.dma_start(out=xh[:, 6:9, 0:512], in_=xr[:, 6:9, 0:512])
                nc.sync.dma_start(out=xh[:, 9:K1, 0:512], in_=xr[:, 9:K1, 0:512])
                for g in range(NG1):
                    nc.scalar.dma_start(out=w1s[:, g], in_=wr[:, g])
                cv = consts.tile([128, NJ1 + 3 * NJ3], F32)
                nc.sync.dma_start(out=cv, in_=cvec[:, :])
                nt1 = cv[:, 0:NJ1]
                nt2 = cv[:, NJ1:NJ1 + NJ3]
                s3s = cv[:, NJ1 + NJ3:NJ1 + 2 * NJ3]
                c3s = cv[:, NJ1 + 2 * NJ3:NJ1 + 3 * NJ3]
                w4s = consts.tile([128, NJ3, NCLS], F16)
                nc.sync.dma_start(
                    out=w4s, in_=w4t.rearrange("p (j c) -> p j c", c=NCLS))
                b4s = consts.tile([NCLS, 1], F32)
                nc.sync.dma_start(out=b4s, in_=b4c[:, :])
                # x n=1 half: needed only after phase 0 (~65us in)
                nc.sync.dma_start(out=xh[:, 0:7, 512:1024],
                                  in_=xr[:, 0:7, 512:1024])
                nc.sync.dma_start(out=xh[:, 7:K1, 512:1024],
                                  in_=xr[:, 7:K1, 512:1024])
                for n in range(NB):
                    nc.vector.memset(lg[n], 0.0)

                # prefetch the first two fc2 weight slabs during fc1
                def w2_slab(jb):
                    wt = w2s.tile([128, NT2, 2, JB * 128], FP8, tag="w2t")
                    w2r = w2p[jb].rearrange("p (t i c) -> p t i c",
                                            i=2, c=JB * 128)
                    for tg in range(NT2 // 3):
                        nc.sync.dma_start(out=wt[:, 3 * tg:3 * tg + 3],
                                          in_=w2r[:, 3 * tg:3 * tg + 3])
                    return wt

                w2_pre = [w2_slab(0), w2_slab(1)]

                for n in range(NB):
                    for g in range(NG1):
                        pss = [psum.tile([128, 512], F32, tag="ps",
                                         name=f"f1_{n}_{g}_{i}")
                               for i in range(G1)]
                        for k in range(K1 - 1):
                            for j3 in range(G1):
                                nc.tensor.matmul(
                                    pss[j3],
                                    lhsT=w1s[:, g, k, j3 * 128:(j3 + 1) * 128],
                                    rhs=xh[:, k, n * 512:(n + 1) * 512],
                                    start=(k == 0),
                                    stop=False,
                                )
                        # packed 32-row remainder tile: the 3 j-tiles run
                        # CONCURRENTLY in distinct PE row-groups (data is
                        # host-replicated at partition offsets 0/32/64).
                        for j3 in range(G1):
                            p0 = 32 * j3
                            nc.tensor.matmul(
                                pss[j3],
                                lhsT=w1s[p0:p0 + 32, g, K1 - 1,
                                         j3 * 128:(j3 + 1) * 128],
                                rhs=xh[p0:p0 + 32, K1 - 1,
                                       n * 512:(n + 1) * 512],
                                start=False,
                                stop=True,
                                tile_position=(p0, 0),
                            )
                        for j3 in range(G1):
                            j = g * G1 + j3
                            nc.scalar.activation(
                                out=a1[:, j // 2, j % 2, n * 512:(n + 1) * 512],
                                in_=pss[j3],
                                func=ACTF.Sign,
                                bias=nt1[:, j:j + 1],
                                scale=1.0,
                            )

            # ---- fc2: fp8 DoubleRow + sign threshold ----
            with (
                tc.tile_pool(name="w3s", bufs=2) as w3s,
                tc.tile_pool(name="h3p", bufs=6) as h3p,
            ):
                # prefetch the first two fc3 weight slabs (scalar queue)
                def w3_slab(jb):
                    wt = w3s.tile([128, NT3, 2, JB * 128], FP8, tag="w3t")
                    w3r = w3p[jb].rearrange("p (t i c) -> p t i c",
                                            i=2, c=JB * 128)
                    for tg in range(NT3 // 4):
                        nc.scalar.dma_start(out=wt[:, 4 * tg:4 * tg + 4],
                                            in_=w3r[:, 4 * tg:4 * tg + 4])
                    return wt

                w3_pre = [w3_slab(0), w3_slab(1)]

                for jb in range(NJ2 // JB):
                    wt = w2_pre[jb] if jb < 2 else w2_slab(jb)
                    for j in range(JB):
                        jj = jb * JB + j
                        for n in range(NB):
                            ps = psum.tile([128, 512], F32, tag="ps")
                            for t in range(NT2):
                                nc.tensor.matmul(
                                    ps,
                                    lhsT=wt[:, t, :, j * 128:(j + 1) * 128],
                                    rhs=a1[:, t, :, n * 512:(n + 1) * 512],
                                    start=(t == 0),
                                    stop=(t == NT2 - 1),
                                    perf_mode=DR,
                                )
                            nc.scalar.activation(
                                out=a2[:, jj // 2, jj % 2, n * 512:(n + 1) * 512],
                                in_=ps,
                                func=ACTF.Sign,
                                bias=nt2[:, jj:jj + 1],
                                scale=1.0,
                            )

                # ---- fc3 (fp8 DR) + bn3/hardtanh + fused fc4 (fp16) ----
                # fc4 MMs are batched per weight slab (4 jj x 2 n emitted
                # back-to-back one slab late): consecutive M=16 matmuls
                # pipeline, amortizing the array switch cost, and the
                # bn3+clip chains get a full slab of lead time.
                pend4 = []

                def flush_fc4():
                    for n in range(NB):
                        for ph3, pjj in pend4:
                            nc.tensor.matmul(
                                lg[n],
                                lhsT=w4s[:, pjj, :],
                                rhs=ph3[:, n * 512:(n + 1) * 512],
                                start=False,
                                stop=(pjj == NJ3 - 1),
                                skip_group_check=True,
                            )
                    pend4.clear()

                for jb in range(NJ3 // JB):
                    wt = w3_pre[jb] if jb < 2 else w3_slab(jb)
                    for j in range(JB):
                        jj = jb * JB + j
                        h3 = h3p.tile([128, BC], F16, tag="h3")
                        for n in range(NB):
                            ps = psum.tile([128, 512], F32, tag="ps")
                            for t in range(NT3):
                                nc.tensor.matmul(
                                    ps,
                                    lhsT=wt[:, t, :, j * 128:(j + 1) * 128],
                                    rhs=a2[:, t, :, n * 512:(n + 1) * 512],
                                    start=(t == 0),
                                    stop=(t == NT3 - 1),
                                    perf_mode=DR,
                                )
                            if j == 0 and n == 1 and pend4:
                                flush_fc4()
                            tmp = h3p.tile([128, 512], F32, tag="bn3tmp")
                            nc.scalar.activation(
                                out=tmp,
                                in_=ps,
                                func=ACTF.Identity,
                                bias=c3s[:, jj:jj + 1],
                                scale=s3s[:, jj:jj + 1],
                            )
                            nc.vector.tensor_scalar(
                                out=h3[:, n * 512:(n + 1) * 512],
                                in0=tmp,
                                scalar1=-1.0,
                                scalar2=1.0,
                                op0=ALU.max,
                                op1=ALU.min,
                            )
                        pend4.append((h3, jj))
                flush_fc4()

            # ---- epilogue: +b4, transpose [cls,b]->[b,cls], log_softmax ----
            with tc.tile_pool(name="epi", bufs=1) as epi:
                ident = consts.tile([NCLS, NCLS], F32)
                make_identity(nc, ident)
                lsb = epi.tile([NCLS, BC], F32, tag="lsb")
                tp = psum_tp.tile([128, NBCH, NCLS], F32, tag="tp")
                # log_softmax without max-shift: logits are O(5), exp safe;
                # exp runs per batch-half as its transposes complete
                ex = epi.tile([128, NBCH, 10], F32, tag="ex")
                for n in range(NB):
                    nc.scalar.activation(
                        out=lsb[:, n * 512:(n + 1) * 512],
                        in_=lg[n],
                        func=ACTF.Identity,
                        bias=b4s[:, 0:1],
                        scale=1.0,
                    )
                    for b in range(4):
                        c = n * 4 + b
                        nc.tensor.transpose(
                            tp[:, c, :], lsb[:, c * 128:(c + 1) * 128], ident)
                    nc.scalar.activation(
                        out=ex[:, n * 4:(n + 1) * 4, :],
                        in_=tp[:, n * 4:(n + 1) * 4, 0:10], func=ACTF.Exp)
                sm = epi.tile([128, NBCH], F32, tag="sm")
                nc.vector.tensor_reduce(
                    out=sm, in_=ex, axis=mybir.AxisListType.X, op=ALU.add)
                lnt = epi.tile([128, NBCH], F32, tag="lnt")
                nc.scalar.activation(out=lnt, in_=sm, func=ACTF.Ln)
                res = epi.tile([128, NBCH, 10], F32, tag="res")
                for b in range(NBCH):
                    nc.vector.tensor_scalar(
                        out=res[:, b, :], in0=tp[:, b, 0:10],
                        scalar1=lnt[:, b:b + 1],
                        scalar2=None, op0=ALU.subtract,
                    )
                nc.sync.dma_start(
                    out=out.rearrange("(c p) f -> p c f", p=128), in_=res)

    _legalize_multiwait(nc)
    return nc


def _prep_inputs(inputs):
    f64 = {k: np.asarray(v, np.float64) for k, v in inputs.items()
           if k != "x"}
    x = np.asarray(inputs["x"], np.float32)

    s1 = f64["g1"] / np.sqrt(f64["v1"] + EPS)
    t1 = f64["m1"] - f64["b1"] - f64["be1"] / s1
    s2 = f64["g2"] / np.sqrt(f64["v2"] + EPS)
    t2 = f64["m2"] - f64["b2"] - f64["be2"] / s2
    s3 = f64["g3"] / np.sqrt(f64["v3"] + EPS)
    c3 = (f64["b3"] - f64["m3"]) * s3 + f64["be3"]

    shared = {}
    # cvec [128, 24+48*3]: per-feature consts arranged [partition, tile]
    cvec = np.zeros((128, NJ1 + 3 * NJ3), np.float32)
    cvec[:, 0:NJ1] = (-t1).astype(np.float32).reshape(NJ1, 128).T
    cvec[:, NJ1:NJ1 + NJ3] = (-t2).astype(np.float32).reshape(NJ3, 128).T
    cvec[:, NJ1 + NJ3:NJ1 + 2 * NJ3] = s3.astype(np.float32).reshape(NJ3, 128).T
    cvec[:, NJ1 + 2 * NJ3:] = c3.astype(np.float32).reshape(NJ3, 128).T
    shared["cvec"] = np.ascontiguousarray(cvec)

    b4p = np.zeros((NCLS, 1), np.float32)
    b4p[:10, 0] = np.asarray(inputs["b4"], np.float32)
    shared["b4c"] = b4p

    # w1: sign, transposed to [in, out]; k-tiles 0-5 = pass1 rows (+-1),
    # 6-11 = pass2 rows (+-2^-11), 12 = packed remainders of both passes.
    # Then permuted to j-group-major so each group is one contiguous DMA.
    w1b = np.sign(np.asarray(inputs["w1"], np.float32)).astype(np.float32)
    w1T = w1b.T  # [784, D1]
    w1f = np.zeros((128, K1, D1), np.float32)
    for k in range(K1F):
        w1f[:, k, :] = w1T[k * 128:(k + 1) * 128]
        w1f[:, k + K1F, :] = w1T[k * 128:(k + 1) * 128] * S2W
    w1f[0:16, 12, :] = w1T[768:784]
    w1f[16:32, 12, :] = w1T[768:784] * S2W
    for r in range(1, 3):  # replicate packed tile for PE row-group tiling
        w1f[32 * r:32 * r + 32, 12, :] = w1f[0:32, 12, :]
    w1e5 = w1f.astype(NP_FP8E5)
    shared["w1t"] = np.ascontiguousarray(
        w1e5.reshape(128, K1, NG1, G1 * 128).transpose(0, 2, 1, 3)
        .reshape(128, NG1 * K1 * G1 * 128))

    # w2/w3: sign -> DoubleRow pair layout, slab-contiguous per partition:
    # [njb, 128, nt*2*(JB*128)] fp8
    def pack_dr(w, njb_out):
        wT = np.sign(np.asarray(w, np.float32)).T  # [in, out]
        nin, nout = wT.shape
        nt = nin // 256
        a = wT.reshape(nt, 2, 128, nout).transpose(0, 2, 1, 3)  # [nt,128,2,out]
        a = a.reshape(nt, 128, 2, njb_out, JB * 128).transpose(3, 1, 0, 2, 4)
        # a: [njb, 128, nt, 2, JB*128]
        return np.ascontiguousarray(
            a.reshape(njb_out, 128, nt * 2 * JB * 128).astype(NP_FP8))

    shared["w2p"] = pack_dr(inputs["w2"], NJ2 // JB)
    shared["w3p"] = pack_dr(inputs["w3"], NJ3 // JB)

    # w4: [10, D2] -> fp16 [128, NJ3*NCLS]: elem [k, j*16+c] = w4[c, j*128+k]
    w4 = np.asarray(inputs["w4"], np.float32)
    w4tp = np.zeros((D2, NCLS), np.float32)
    w4tp[:, :10] = w4.T
    shared["w4t"] = np.ascontiguousarray(
        w4tp.reshape(NJ3, 128, NCLS).transpose(1, 0, 2)
        .reshape(128, NJ3 * NCLS).astype(np.float16))

    # x: transpose, fp16x2 split (pass2 scaled by 2^11); per-core layout
    # [128, K1*BC] with k-tile-major columns.
    xT = np.ascontiguousarray(x.T)  # [784, B]
    x1 = xT.astype(np.float16)
    x2s = ((xT - x1.astype(np.float32)) * S2L).astype(np.float16)
    per_core = []
    for cix in range(NCORES):
        sl = slice(cix * BC, (cix + 1) * BC)
        xa = np.zeros((K1, 128, BC), np.float16)
        xa[0:K1F] = x1[0:768, sl].reshape(K1F, 128, BC)
        xa[K1F:2 * K1F] = x2s[0:768, sl].reshape(K1F, 128, BC)
        xa[12, 0:16] = x1[768:784, sl]
        xa[12, 16:32] = x2s[768:784, sl]
        for r in range(1, 3):
            xa[12, 32 * r:32 * r + 32] = xa[12, 0:32]
        m = dict(shared)
        m["xht"] = np.ascontiguousarray(
            xa.transpose(1, 0, 2).reshape(128, K1 * BC))
        per_core.append(m)
    return per_core


_NC_CACHE = None


def kernel(**inputs):
    global _NC_CACHE, LAST_EXEC_NS
    if _NC_CACHE is None:
        _NC_CACHE = _build_nc()
    nc = _NC_CACHE
    in_maps = _prep_inputs(inputs)
    kwargs = {}
    if TRACE:
        _install_ntff_shim()
        kwargs = dict(trace=True, tmpdir=TRACE_DIR)
    res = None
    outs = None
    for attempt in range(3):
        try:
            res = run_bass_kernel_spmd(nc, in_maps, core_ids=list(range(NCORES)),
                                       **kwargs)
            outs = [np.asarray(res.results[c]["out"]) for c in range(NCORES)]
            break
        except Exception:
            if attempt == 2:
                raise
    LAST_EXEC_NS = res.exec_time_ns
    return np.concatenate(outs, axis=0)


def _install_ntff_shim():
    """antenv.axon_hooks shim so trace=True works under axon (profiling only)."""
    import contextlib
    import ctypes
    import types

    if "antenv.axon_hooks" in sys.modules:
        return
    try:
        lib = ctypes.CDLL("/opt/axon/libaxon_pjrt.so")
        lib.axon_start_nrt_profile.argtypes = [
            ctypes.POINTER(ctypes.c_int64), ctypes.c_size_t]
        lib.axon_start_nrt_profile.restype = ctypes.c_int64
        lib.axon_stop_nrt_profile.argtypes = [ctypes.c_char_p]
        lib.axon_stop_nrt_profile.restype = ctypes.c_int64
    except (OSError, AttributeError):
        return

    @contextlib.contextmanager
    def _hook(output_dir, device_ids):
        import jax
        jax.devices()
        if device_ids:
            ids = (ctypes.c_int64 * len(device_ids))(*device_ids)
            rc = lib.axon_start_nrt_profile(ids, len(device_ids))
        else:
            rc = lib.axon_start_nrt_profile(None, 0)
        if rc != 0:
            raise RuntimeError(f"axon_start_nrt_profile rc={rc}")
        try:
            yield
        finally:
            n = lib.axon_stop_nrt_profile(str(output_dir).encode())
            print(f"ntff: {n} profile file(s) -> {output_dir}", file=sys.stderr)

    mod = types.ModuleType("antenv.axon_hooks")
    mod.get_axon_ntff_profile_hook = lambda: _hook
    mod.set_axon_ntff_profile_hook = lambda h: None
    sys.modules["antenv.axon_hooks"] = mod
